# revision 1
# baseline (speedup 1.0000x reference)
"""Trainium2 Bass kernel for nn_CellEncoder (2-layer GraphSAGE, mean aggregation).

Strategy (8 NeuronCores, SPMD, node-partitioned), v2 (bf16 + 4-queue gathers):
  - Core c owns nodes [c*npc, (c+1)*npc).  Aggregation is linear, so the
    dense transform is applied FIRST: z = h @ W_l.T reduces gather width
    from in_dim (1000) to emb (128) values per edge.  All tables/operands
    are bf16 (PSUM accumulation fp32); tolerance is 2e-2, bf16 ~5e-3.
  - Per layer: each core computes z for its own nodes, contributes two
    half-slabs to two AllGathers forming table_lo/table_hi (rows < 32768
    so int16 dma_gather indices address them at 256B stride).
  - Edges grouped by dst tile (128 dsts); each tile's edges packed into
    128-slot chunks (lo chunks then hi chunks).  dma_gather pulls slot
    rows into SBUF.  4 SWDGE queues are used round-robin: queue q is
    served by Q7 cores {2q, 2q+1}, so 4 in-flight gathers emit
    descriptors in parallel (the previous 2-in-flight structure made Q7
    descriptor generation the critical path at ~6.5ns/edge).
  - One-hot scatter matrices S[e,d] = (dst(e)==d) are built on DVE with a
    single batched is_equal per st-group (stride-0 broadcast APs).
    PE accumulates aggT[f,d] += G_chunk.T @ S_chunk in PSUM.
  - ELU's "-1" is folded out: the device computes h~ = elu(x)+1
    (= max(x,0)+exp(min(x,0))); the next layer's bias is adjusted on the
    host (b1' = b1 - W_l1@1 - W_r1@1) and the host subtracts 1 from the
    final output.  Saves a DVE op per group (requires min degree >= 1,
    checked on host).
  - Epilogue feature-major; Relu/Exp run on the scalar engine, the rest
    on DVE in bf16.  Output written bf16 [128, NPAD]; host casts,
    subtracts 1, transposes, trims.

kernel(**inputs) takes FULL inputs, shards internally, runs one NEFF on
cores 0-7 via bass_utils.run_bass_kernel_spmd, returns the full output.
"""
import os
import sys

import numpy as np

for _p in ("/opt/trn_rl_repo", "/root/.axon_site/_ro/trn_rl_repo"):
    if os.path.isdir(_p) and _p not in sys.path:
        sys.path.append(_p)

import ml_dtypes

import concourse.bass as bass
import concourse.bacc as bacc
import concourse.mybir as mybir
import concourse.tile as tile
from concourse import bass_utils

P = 128
F32 = mybir.dt.float32
BF16 = mybir.dt.bfloat16
AF = mybir.ActivationFunctionType
ALU = mybir.AluOpType

# SWDGE descriptor-ring sizing: ring holds scratch//16 descriptors; one
# dma_gather must fit in its queue's ring.  Measured on HW: >1024 idxs needs
# single_packet=False.  Keep gathers <= GMAX chunks so they fit the ring.
SCRATCH = 49152
# chunks (1024 idxs) per dma_gather: <=1024 idxs takes the single_packet
# fast path and 3 gathers fit in one queue's 3072-desc ring, so same-queue
# emission pipelines instead of serializing emit->drain (Tile inserts
# ring-reuse waits on the DMA sem of the gather 2 ring-slots back).
# Measured: 1024-idx gathers beat both 768 (per-gather overhead) and
# 2816 (ring-reuse serialization).
GMAX = 8


def build_meta(N, NC, dst, src, tiles_per_st):
    """Static chunk structure (shared across cores; max-over-core sizes) and
    per-core gather-index / dst-id slabs."""
    npc = N // NC
    half = npc // 2
    TPC = (npc + P - 1) // P
    NPAD = TPC * P
    NST = (TPC + tiles_per_st - 1) // tiles_per_st

    c = dst // npc
    d = (dst - c * npc).astype(np.int64)
    t = d // P
    did = d % P
    sc = src // npc
    sp = src - sc * npc
    tb = (sp >= half).astype(np.int64)
    row = sc * half + np.where(tb == 0, sp, sp - half)
    assert row.max() < 32768

    nlohi = np.zeros((NC, TPC, 2), np.int64)
    np.add.at(nlohi, (c, t, tb), 1)
    KL = np.maximum(1, (nlohi[:, :, 0].max(axis=0) + P - 1) // P)
    KH = ((nlohi[:, :, 1].max(axis=0) + P - 1) // P).astype(np.int64)

    Ktot = KL + KH
    chunk_base = np.concatenate([[0], np.cumsum(Ktot)])
    NCHUNK = int(chunk_base[-1])

    st_tiles = [list(range(s * tiles_per_st, min((s + 1) * tiles_per_st, TPC)))
                for s in range(NST)]
    GL = [int(sum(KL[tt] for tt in ts)) for ts in st_tiles]
    GH = [int(sum(KH[tt] for tt in ts)) for ts in st_tiles]

    idx_off_lo, idx_off_hi = [], []
    off = 0
    for s in range(NST):
        idx_off_lo.append(off); off += GL[s] * P // 16
        idx_off_hi.append(off); off += GH[s] * P // 16
    NIDX16 = off

    idx_slab = np.zeros((NC, P, NIDX16), np.int16)
    dstid_slab = np.full((NC, P, NCHUNK), -1.0, np.float32)
    cnt = np.zeros((NC, NPAD), np.int64)

    order = np.lexsort((tb, t, c))
    co, to, tbo = c[order], t[order], tb[order]
    rowo, dido, do_ = row[order], did[order], d[order]
    np.add.at(cnt, (co, do_), 1)

    key = (co * TPC + to) * 2 + tbo
    bounds = np.concatenate([[0], np.nonzero(np.diff(key))[0] + 1, [len(key)]])
    gval_lo = [np.zeros((NC, GL[s] * P), np.int16) for s in range(NST)]
    gval_hi = [np.zeros((NC, GH[s] * P), np.int16) for s in range(NST)]

    lo_base = np.zeros(TPC, np.int64)
    hi_base = np.zeros(TPC, np.int64)
    for s, ts in enumerate(st_tiles):
        accl = acch = 0
        for tt in ts:
            lo_base[tt] = accl; accl += KL[tt] * P
            hi_base[tt] = acch; acch += KH[tt] * P

    for bi in range(len(bounds) - 1):
        lo_, hi_ = int(bounds[bi]), int(bounds[bi + 1])
        if lo_ == hi_:
            continue
        cc, tt, bb = int(co[lo_]), int(to[lo_]), int(tbo[lo_])
        n = hi_ - lo_
        s = tt // tiles_per_st
        if bb == 0:
            base = int(lo_base[tt])
            gval_lo[s][cc, base:base + n] = rowo[lo_:hi_]
            ch0 = int(chunk_base[tt])
        else:
            base = int(hi_base[tt])
            gval_hi[s][cc, base:base + n] = rowo[lo_:hi_]
            ch0 = int(chunk_base[tt]) + int(KL[tt])
        # base is a multiple of P: slot partition (base+i)%P == i%P and
        # gather block base//P + i//P lines up with tile chunk ch0 + i//P.
        local = np.arange(n)
        dstid_slab[cc, local % P, ch0 + local // P] = dido[lo_:hi_]

    for s in range(NST):
        for cc in range(NC):
            for vals, o in ((gval_lo[s][cc], idx_off_lo[s]),
                            (gval_hi[s][cc], idx_off_hi[s])):
                n = len(vals)
                if n == 0:
                    continue
                w = vals.reshape(n // 16, 16).T
                idx_slab[cc, :, o:o + n // 16] = np.tile(w, (8, 1))

    inv = (1.0 / np.maximum(cnt, 1)).astype(np.float32)
    # nodes with zero in-degree would break the h~=elu+1 bias-shift trick
    # (padding nodes [npc:NPAD) never enter the z tables, so only real ones count)
    shift_ok = bool(cnt[:, :npc].min() >= 1)

    return dict(
        npc=npc, half=half, TPC=TPC, NPAD=NPAD, NST=NST, st_tiles=st_tiles,
        KL=[int(v) for v in KL], KH=[int(v) for v in KH],
        chunk_base=[int(v) for v in chunk_base], NCHUNK=NCHUNK,
        GL=GL, GH=GH, idx_off_lo=idx_off_lo, idx_off_hi=idx_off_hi,
        NIDX16=NIDX16, idx_slab=idx_slab, dstid_slab=dstid_slab, inv=inv,
        shift_ok=shift_ok,
    )


# ---------------------------------------------------------------------------
# device kernel builder
# ---------------------------------------------------------------------------

def build_kernel(meta, in_dim, NC):
    npc, half = meta["npc"], meta["half"]
    TPC, NPAD, NST = meta["TPC"], meta["NPAD"], meta["NST"]
    NCHUNK, NIDX16 = meta["NCHUNK"], meta["NIDX16"]
    KL, KH, chunk_base = meta["KL"], meta["KH"], meta["chunk_base"]
    shift = meta["shift_ok"]
    GC = (in_dim + P - 1) // P
    GPAD = GC * P
    WMAX = max(len(ts) for ts in meta["st_tiles"]) * P
    gq = [0]  # gather queue round-robin over all 4 SWDGE queues

    nc = bacc.Bacc("TRN2", target_bir_lowering=False, debug=False,
                   enable_asserts=False, num_devices=NC,
                   dynamic_dma_scratch_size=SCRATCH, num_swdge_queues=4)

    x_d = nc.dram_tensor("x_pad", [GPAD, NPAD], BF16, kind="ExternalInput").ap()
    w0l_d = nc.dram_tensor("W0lT", [GPAD, P], BF16, kind="ExternalInput").ap()
    w0r_d = nc.dram_tensor("W0rT", [GPAD, P], BF16, kind="ExternalInput").ap()
    w1l_d = nc.dram_tensor("W1lT", [P, P], BF16, kind="ExternalInput").ap()
    w1r_d = nc.dram_tensor("W1rT", [P, P], BF16, kind="ExternalInput").ap()
    b0_d = nc.dram_tensor("b0col", [P, 1], F32, kind="ExternalInput").ap()
    b1_d = nc.dram_tensor("b1col", [P, 1], F32, kind="ExternalInput").ap()
    inv_d = nc.dram_tensor("invt", [P, NPAD], F32, kind="ExternalInput").ap()
    idx_d = nc.dram_tensor("idx16", [P, NIDX16], mybir.dt.int16,
                           kind="ExternalInput").ap()
    iota_d = nc.dram_tensor("iota", [P, P], BF16, kind="ExternalInput").ap()
    dst_d = nc.dram_tensor("dstid", [P, NCHUNK], BF16, kind="ExternalInput").ap()
    out_d = nc.dram_tensor("outT", [P, NPAD], BF16, kind="ExternalOutput").ap()

    with tile.TileContext(nc, num_cores=NC) as tc:
        with (
            tc.tile_pool(name="const", bufs=1) as cpool,
            tc.tile_pool(name="slab", bufs=1) as slab,
            tc.tile_pool(name="zp", bufs=3) as zpool,
            tc.tile_pool(name="ep", bufs=2) as epool,
            tc.tile_pool(name="pz", bufs=3, space="PSUM") as pz,
            tc.tile_pool(name="pr", bufs=2, space="PSUM") as pr,
            tc.tile_pool(name="pa", bufs=2, space="PSUM") as pa,
            tc.tile_pool(name="dram", bufs=1, space="DRAM") as dram,
        ):
            # ---- constants ----
            w0l_sb = cpool.tile([P, GC * P], BF16)
            w0r_sb = cpool.tile([P, GC * P], BF16)
            for gc in range(GC):
                nc.sync.dma_start(out=w0l_sb[:, gc * P:(gc + 1) * P],
                                  in_=w0l_d[gc * P:(gc + 1) * P, :])
                nc.sync.dma_start(out=w0r_sb[:, gc * P:(gc + 1) * P],
                                  in_=w0r_d[gc * P:(gc + 1) * P, :])
            w1l_sb = cpool.tile([P, P], BF16)
            nc.sync.dma_start(out=w1l_sb[:], in_=w1l_d[:])
            w1r_sb = cpool.tile([P, P], BF16)
            nc.sync.dma_start(out=w1r_sb[:], in_=w1r_d[:])
            b0_sb = cpool.tile([P, 1], F32)
            nc.sync.dma_start(out=b0_sb[:], in_=b0_d[:])
            b1_sb = cpool.tile([P, 1], F32)
            nc.sync.dma_start(out=b1_sb[:], in_=b1_d[:])
            zero_sb = cpool.tile([P, 1], BF16)
            nc.vector.memset(zero_sb[:], 0.0)
            mone_sb = cpool.tile([P, 1], BF16)
            nc.vector.memset(mone_sb[:], -1.0)
            iota_sb = cpool.tile([P, P], BF16)
            nc.sync.dma_start(out=iota_sb[:], in_=iota_d[:])
            dst_sb = cpool.tile([P, NCHUNK], BF16)
            nc.sync.dma_start(out=dst_sb[:], in_=dst_d[:])
            IDXW = max(meta["GL"][s] + meta["GH"][s] for s in range(NST)) * 8
            SMAX = max(meta["GL"][s] + meta["GH"][s] for s in range(NST)) * P

            rb0_sb = slab.tile([P, NPAD], BF16)
            rb1_sb = slab.tile([P, NPAD], BF16)

            # ---- collective buffers ----
            def cc_pair(nm):
                i_lo = dram.tile([half, P], BF16, name=f"cci_lo{nm}")
                i_hi = dram.tile([half, P], BF16, name=f"cci_hi{nm}")
                o_lo = dram.tile([NC * half, P], BF16, addr_space="Shared",
                                 name=f"cco_lo{nm}")
                o_hi = dram.tile([NC * half, P], BF16, addr_space="Shared",
                                 name=f"cco_hi{nm}")
                return i_lo, i_hi, o_lo, o_hi

            cc0 = cc_pair("0")
            cc1 = cc_pair("1")
            rg = [list(range(NC))]

            # result writes go on the Activation engine's HWDGE stream so the
            # Sync engine stays a pure prefetch FIFO (a result write depends on
            # the whole gather->matmul->epilogue chain; putting it on Sync
            # head-of-line blocks the next group's idx/inv prefetch loads)
            def z_to_cc(z_sb, tt, cc):
                r0, r1 = tt * P, min(tt * P + P, npc)
                for lo_s, hi_s, tgt, base in (
                        (r0, min(r1, half), cc[0], 0),
                        (max(r0, half), r1, cc[1], half)):
                    if hi_s > lo_s:
                        nc.scalar.dma_start(
                            out=tgt[lo_s - base:hi_s - base, :],
                            in_=z_sb[lo_s - r0:hi_s - r0, :])

            # all lo-half z rows are written once st-group s_ag is done; the
            # lo AllGather trigger goes right after it in the gpsimd FIFO so
            # it is not stuck behind later gather instructions
            s_ag = ((half + P - 1) // P - 1) // len(meta["st_tiles"][0])

            def ag(cc, which):
                nc.gpsimd.collective_compute(
                    "AllGather", ALU.bypass, replica_groups=rg,
                    ins=[cc[which][:].opt()], outs=[cc[which + 2][:].opt()])

            # ---- phase A: z0 (node-major) + rb0T (feature-major) ----
            # x tiles are phase-A-only; scoping the pool frees its SBUF for
            # the aggregation-phase pools
            with tc.tile_pool(name="xp", bufs=2) as xpool:
                for s, ts in enumerate(meta["st_tiles"]):
                    w = len(ts) * P
                    c0 = ts[0] * P
                    xg = xpool.tile([P, GC * w], BF16, tag="xg",
                                    padded_shape=[P, GC * WMAX])
                    nc.sync.dma_start(
                        out=xg[:].rearrange("p (gc j) -> p gc j", gc=GC),
                        in_=x_d[:, c0:c0 + w].rearrange("(gc p) j -> p gc j", p=P))
                    r0ps = pr.tile([P, w], F32, tag="rps", padded_shape=[P, WMAX])
                    for gc in range(GC):
                        nc.tensor.matmul(out=r0ps[:],
                                         lhsT=w0r_sb[:, gc * P:(gc + 1) * P],
                                         rhs=xg[:, gc * w:(gc + 1) * w],
                                         start=(gc == 0), stop=(gc == GC - 1))
                    nc.vector.tensor_tensor(out=rb0_sb[:, c0:c0 + w], in0=r0ps[:],
                                            in1=b0_sb[:, :1].to_broadcast([P, w]),
                                            op=ALU.add)
                    for ti, tt in enumerate(ts):
                        z0ps = pz.tile([P, P], F32, tag="zps")
                        for gc in range(GC):
                            nc.tensor.matmul(
                                out=z0ps[:],
                                lhsT=xg[:, gc * w + ti * P:gc * w + (ti + 1) * P],
                                rhs=w0l_sb[:, gc * P:(gc + 1) * P],
                                start=(gc == 0), stop=(gc == GC - 1))
                        z0sb = zpool.tile([P, P], BF16, tag="zsb")
                        nc.vector.tensor_copy(out=z0sb[:], in_=z0ps[:])
                        z_to_cc(z0sb, tt, cc0)
                    if s == s_ag:
                        ag(cc0, 0)
            ag(cc0, 1)

            with (
                tc.tile_pool(name="gat", bufs=4) as gpool,
                tc.tile_pool(name="sp", bufs=2) as spool,
                tc.tile_pool(name="ip", bufs=2) as ipool,
                tc.tile_pool(name="xip", bufs=4) as xipool,
            ):
                def gather_split(table, nch, idx_sb, idx_off, tag):
                    """One or more dma_gathers (<= GMAX chunks each) into one
                    SBUF tile [P, nch*P], rotating across all 4 SWDGE queues."""
                    if nch == 0:
                        return None
                    g = gpool.tile([P, nch * P], BF16, tag=tag,
                                   padded_shape=[P, max(max(meta["GL"]), max(meta["GH"])) * P])
                    done = 0
                    while done < nch:
                        n = min(GMAX, nch - done)
                        gq[0] = (gq[0] + 1) % 4
                        nc.gpsimd.dma_gather(
                            out_ap=g[:, done * P:(done + n) * P]
                            .rearrange("p (k e) -> p k e", e=P),
                            in_ap=table[:],
                            idxs_ap=idx_sb[:, idx_off + done * 8:
                                           idx_off + (done + n) * 8],
                            num_idxs=n * P, num_idxs_reg=n * P, elem_size=P,
                            single_packet=(n * P <= 1024), queue_num=gq[0])
                        done += n
                    return g

                # ---- aggregation + epilogue (shared for both layers) ----
                # lo gathers are issued LK groups ahead of hi gathers: the hi
                # table's AllGather finishes after the lo one, and a hi gather
                # stuck waiting on it head-of-line blocks every later gather
                # in the GpSimd FIFO; front-loading lo work fills that window
                def issue_lo(s, tables):
                    GLs, GHs = meta["GL"][s], meta["GH"][s]
                    nch_st = GLs + GHs
                    idxt = xipool.tile([P, nch_st * 8], mybir.dt.int16, tag="idxt",
                                       padded_shape=[P, IDXW])
                    o_lo = meta["idx_off_lo"][s]
                    nc.sync.dma_start(out=idxt[:],
                                      in_=idx_d[:, o_lo:o_lo + nch_st * 8])
                    glo = gather_split(tables[0], GLs, idxt, 0, "glo")
                    return idxt, glo

                def aggregate(s, ts, tables, rb_slab, out_cb, pre):
                    w = len(ts) * P
                    c0 = ts[0] * P
                    GLs, GHs = meta["GL"][s], meta["GH"][s]
                    nch_st = GLs + GHs
                    cb0 = chunk_base[ts[0]]
                    idxt, glo = pre
                    ghi = gather_split(tables[1], GHs, idxt, GLs * 8, "ghi")
                    # batched one-hot build: S[slot, cg*P + d] = (dstid[slot,cg]==d)
                    s_sb = spool.tile([P, nch_st * P], BF16, tag="ssb",
                                      padded_shape=[P, SMAX])
                    nc.vector.tensor_tensor(
                        out=s_sb[:].rearrange("p (n e) -> p n e", e=P),
                        in0=dst_sb[:, cb0:cb0 + nch_st].unsqueeze(2)
                        .to_broadcast([P, nch_st, P]),
                        in1=iota_sb[:].unsqueeze(1).to_broadcast([P, nch_st, P]),
                        op=ALU.is_equal)
                    aggps = pa.tile([P, w], F32, tag="aggps", padded_shape=[P, WMAX])
                    lo_blk = hi_blk = 0
                    for ti, tt in enumerate(ts):
                        nch = KL[tt] + KH[tt]
                        for j in range(nch):
                            cg = chunk_base[tt] + j - cb0
                            if j < KL[tt]:
                                g_ap = glo[:, (lo_blk + j) * P:(lo_blk + j + 1) * P]
                            else:
                                jj = j - KL[tt]
                                g_ap = ghi[:, (hi_blk + jj) * P:(hi_blk + jj + 1) * P]
                            nc.tensor.matmul(out=aggps[:, ti * P:(ti + 1) * P],
                                             lhsT=g_ap,
                                             rhs=s_sb[:, cg * P:(cg + 1) * P],
                                             start=(j == 0), stop=(j == nch - 1))
                        lo_blk += KL[tt]
                        hi_blk += KH[tt]
                    # epilogue: x = aggT*inv + rb ; h~ = max(x,0) + exp(min(x,0))
                    invt = ipool.tile([P, w], F32, tag="invt",
                                      padded_shape=[P, WMAX])
                    nc.sync.dma_start(out=invt[:], in_=inv_d[:, c0:c0 + w])
                    x2 = epool.tile([P, w], BF16, tag="x2", padded_shape=[P, WMAX])
                    nc.vector.tensor_tensor(out=x2[:], in0=aggps[:],
                                            in1=invt[:], op=ALU.mult)
                    x3 = epool.tile([P, w], BF16, tag="x3", padded_shape=[P, WMAX])
                    nc.vector.tensor_tensor(out=x3[:], in0=x2[:],
                                            in1=rb_slab[:, c0:c0 + w], op=ALU.add)
                    xm = epool.tile([P, w], BF16, tag="xm", padded_shape=[P, WMAX])
                    nc.scalar.activation(out=xm[:], in_=x3[:], func=AF.Relu)
                    xc = epool.tile([P, w], BF16, tag="xc", padded_shape=[P, WMAX])
                    nc.vector.tensor_tensor(out=xc[:], in0=x3[:],
                                            in1=zero_sb[:, :1].to_broadcast([P, w]),
                                            op=ALU.min)
                    xe = epool.tile([P, w], BF16, tag="xe", padded_shape=[P, WMAX])
                    nc.scalar.activation(out=xe[:], in_=xc[:], func=AF.Exp)
                    h = epool.tile([P, w], BF16, tag="h", padded_shape=[P, WMAX])
                    nc.vector.tensor_tensor(out=h[:], in0=xm[:], in1=xe[:],
                                            op=ALU.add)
                    if not shift:
                        h2 = epool.tile([P, w], BF16, tag="h2",
                                        padded_shape=[P, WMAX])
                        nc.vector.tensor_tensor(
                            out=h2[:], in0=h[:],
                            in1=mone_sb[:, :1].to_broadcast([P, w]), op=ALU.add)
                        h = h2
                    out_cb(s, ts, w, c0, h)

                # ---- phase B+C: layer-0 aggregate -> h1T -> z1/rb1T ----
                def l0_out(s, ts, w, c0, h):
                    for ti, tt in enumerate(ts):
                        z1ps = pz.tile([P, P], F32, tag="zps")
                        nc.tensor.matmul(out=z1ps[:],
                                         lhsT=h[:, ti * P:(ti + 1) * P],
                                         rhs=w1l_sb[:], start=True, stop=True)
                        z1sb = zpool.tile([P, P], BF16, tag="zsb")
                        nc.vector.tensor_copy(out=z1sb[:], in_=z1ps[:])
                        z_to_cc(z1sb, tt, cc1)
                    r1ps = pr.tile([P, w], F32, tag="rps", padded_shape=[P, WMAX])
                    nc.tensor.matmul(out=r1ps[:], lhsT=w1r_sb[:], rhs=h[:],
                                     start=True, stop=True)
                    nc.vector.tensor_tensor(out=rb1_sb[:, c0:c0 + w], in0=r1ps[:],
                                            in1=b1_sb[:, :1].to_broadcast([P, w]),
                                            op=ALU.add)

                LK = 3  # lo-gather lookahead over hi gathers (= xip/gat bufs - 1)

                def run_layer(tables, rb_slab, out_cb, mid_cb=None):
                    pre = {}
                    for s in range(min(LK, NST)):
                        pre[s] = issue_lo(s, tables)
                    for s, ts in enumerate(meta["st_tiles"]):
                        if s + LK < NST:
                            pre[s + LK] = issue_lo(s + LK, tables)
                        aggregate(s, ts, tables, rb_slab, out_cb, pre.pop(s))
                        if mid_cb is not None and s == s_ag:
                            mid_cb()

                run_layer((cc0[2], cc0[3]), rb0_sb, l0_out,
                          mid_cb=lambda: ag(cc1, 0))
                ag(cc1, 1)

                # ---- phase D: layer-1 aggregate -> output ----
                def l1_out(s, ts, w, c0, h):
                    nc.scalar.dma_start(out=out_d[:, c0:c0 + w], in_=h[:])

                run_layer((cc1[2], cc1[3]), rb1_sb, l1_out)

    nc.compile()
    return nc


# ---------------------------------------------------------------------------
# entry point
# ---------------------------------------------------------------------------

def _bf16(a):
    return np.asarray(a, np.float32).astype(ml_dtypes.bfloat16)


def _prepare(x, knn_edge_index, W_l0, b_l0, W_r0, W_l1, b_l1, W_r1,
             NC=8, tiles_per_st=4):
    x = np.asarray(x, np.float32)
    e = np.asarray(knn_edge_index)
    in_dim, N = x.shape
    src, dst = e[0].astype(np.int64), e[1].astype(np.int64)
    meta = build_meta(N, NC, dst, src, tiles_per_st)
    npc, NPAD = meta["npc"], meta["NPAD"]
    GC = (in_dim + P - 1) // P
    GPAD = GC * P

    W_l1 = np.asarray(W_l1, np.float32)
    W_r1 = np.asarray(W_r1, np.float32)
    b1 = np.asarray(b_l1, np.float32)
    if meta["shift_ok"]:
        b1 = b1 - W_l1.sum(axis=1) - W_r1.sum(axis=1)

    w0l = np.zeros((GPAD, P), np.float32); w0l[:in_dim] = np.asarray(W_l0).T
    w0r = np.zeros((GPAD, P), np.float32); w0r[:in_dim] = np.asarray(W_r0).T
    shared = {
        "W0lT": _bf16(w0l), "W0rT": _bf16(w0r),
        "W1lT": _bf16(np.ascontiguousarray(W_l1.T)),
        "W1rT": _bf16(np.ascontiguousarray(W_r1.T)),
        "b0col": np.asarray(b_l0, np.float32).reshape(P, 1),
        "b1col": b1.reshape(P, 1),
        "iota": _bf16(np.broadcast_to(np.arange(P, dtype=np.float32), (P, P))),
    }
    in_maps = []
    for c in range(NC):
        xp = np.zeros((GPAD, NPAD), np.float32)
        xp[:in_dim, :npc] = x[:, c * npc:(c + 1) * npc]
        m = dict(shared)
        m["x_pad"] = _bf16(xp)
        m["invt"] = np.ascontiguousarray(
            np.broadcast_to(meta["inv"][c], (P, NPAD)))
        m["idx16"] = np.ascontiguousarray(meta["idx_slab"][c])
        m["dstid"] = _bf16(meta["dstid_slab"][c])
        in_maps.append(m)
    return meta, in_dim, in_maps


def run(inputs, NC=8, tiles_per_st=4, trace=False, **run_kwargs):
    meta, in_dim, in_maps = _prepare(**inputs, NC=NC, tiles_per_st=tiles_per_st)
    nc = build_kernel(meta, in_dim, NC)
    res = bass_utils.run_bass_kernel_spmd(
        nc, in_maps, core_ids=list(range(NC)), trace=trace, **run_kwargs)
    npc = meta["npc"]
    sub = 1.0 if meta["shift_ok"] else 0.0
    out = np.concatenate(
        [res.results[c]["outT"][:, :npc].T.astype(np.float32) - sub
         for c in range(NC)], axis=0)
    return np.ascontiguousarray(out), res


def kernel(**inputs) -> np.ndarray:
    out, _ = run(inputs)
    return out



# revision 6
# speedup vs baseline: 1.0092x; 1.0092x over previous
"""Trainium2 Bass kernel for nn_CellEncoder (2-layer GraphSAGE, mean aggregation).

Strategy (8 NeuronCores, SPMD, node-partitioned), v3:
  - Core c owns nodes [c*npc, (c+1)*npc).  Aggregation is linear, so the
    dense transform is applied FIRST: z = h @ W_l.T reduces gather width
    from in_dim (1000) to emb (128) values per edge.  All tables/operands
    are bf16 (PSUM accumulation fp32); tolerance is 2e-2, bf16 ~5e-3.
  - Per layer: each core computes z for its own nodes, contributes two
    half-slabs to two AllGathers forming table_lo/table_hi (rows < 32768
    so int16 dma_gather indices address them at 256B stride).
  - Edges grouped by (dst tile, half); slots packed into 128-slot chunks.
    dma_gather pulls slot rows into SBUF on 4 SWDGE queues round-robin.
    v3: gathers are per-(tile,half) pieces of <= 8 chunks with padding
    slots as NEGATIVE indices at the gather tail -- the DMA skips them --
    and num_idxs_reg is reg_load-ed from a per-core count table, so the
    descriptor stream is exactly the core's real edge count (v2 gathered
    ~12% padding descriptors: max-over-core chunk rounding).
  - The int16 index slab for a layer (both layers share it -- same edges)
    is loaded ONCE into SBUF during phase A; the gather stream's only
    dependencies are the z-table AllGathers and gather-buffer reuse.
    Gather buffers are a fixed SBUF slab with manual cycling (NBUF_LO
    lo-tiles in flight, NBUF_HI hi-tiles); lo gathers run ahead to cover
    the AG-hi window, and layer-1 lo gathers are issued during layer-0's
    tail tiles once AG1-lo has completed.
  - One-hot scatter matrices S[e,d] = (dst(e)==d) built on DVE per tile
    with a single batched is_equal (stride-0 broadcast APs).  PE
    accumulates aggT[f,d] += G_chunk.T @ S_chunk in PSUM per group of 4
    tiles.  Skipped padding slots hold zeros/stale z rows (finite) and
    S = 0 there, so they contribute nothing.
  - Phase A computes z0 feature-major with stationary W_l0 (8 LDWEIGHTS
    per 2-tile group instead of 8 per tile), then transposes each tile to
    node-major on the PE against an identity matrix.
  - ELU's "-1" is folded out: the device computes h~ = elu(x)+1
    (= max(x,0)+exp(min(x,0))); the next layer's bias is adjusted on the
    host (b1' = b1 - W_l1@1 - W_r1@1) and the host subtracts 1 from the
    final output.  (requires min in-degree >= 1, checked on host).
  - Epilogue feature-major; Relu/Exp on the scalar engine, rest on DVE.
    Output written bf16 [128, NPAD]; host casts, subtracts 1, transposes,
    trims.

kernel(**inputs) takes FULL inputs, shards internally, runs one NEFF on
cores 0-7 via bass_utils.run_bass_kernel_spmd, returns the full output.
"""
import os
import sys

import numpy as np

for _p in ("/opt/trn_rl_repo", "/root/.axon_site/_ro/trn_rl_repo"):
    if os.path.isdir(_p) and _p not in sys.path:
        sys.path.append(_p)

import ml_dtypes

import concourse.bass as bass
import concourse.bacc as bacc
import concourse.mybir as mybir
import concourse.tile as tile
from concourse import bass_utils

P = 128
F32 = mybir.dt.float32
BF16 = mybir.dt.bfloat16
I16 = mybir.dt.int16
I32 = mybir.dt.int32
AF = mybir.ActivationFunctionType
ALU = mybir.AluOpType

# SWDGE descriptor-ring sizing: ring holds scratch//16 descriptors per queue;
# <=1024-idx gathers take the single_packet fast path and 3 fit in a ring.
SCRATCH = 49152
PIECE = 8          # chunks per gather piece (1024 idxs)
NBUF_HI = 6        # hi-gather tile buffers in flight
HI_LOOK = 4        # hi gather issue lead (tiles); must be < NBUF_HI
L1_GATE = 40       # layer-0 tile index from which layer-1 lo gathers issue


def build_meta(N, NC, dst, src, tiles_per_st):
    """Static chunk structure (shared across cores; max-over-core sizes),
    per-core gather-index / dst-id slabs with -1 padding tails, and per-core
    per-gather-piece valid-index counts."""
    npc = N // NC
    half = npc // 2
    TPC = (npc + P - 1) // P
    NPAD = TPC * P
    NST = (TPC + tiles_per_st - 1) // tiles_per_st

    c = dst // npc
    d = (dst - c * npc).astype(np.int64)
    t = d // P
    did = d % P
    sc = src // npc
    sp = src - sc * npc
    tb = (sp >= half).astype(np.int64)
    row = sc * half + np.where(tb == 0, sp, sp - half)
    assert row.max() < 32768

    nlohi = np.zeros((NC, TPC, 2), np.int64)
    np.add.at(nlohi, (c, t, tb), 1)
    KL = np.maximum(1, (nlohi[:, :, 0].max(axis=0) + P - 1) // P)
    KH = np.maximum(1, (nlohi[:, :, 1].max(axis=0) + P - 1) // P)

    Ktot = KL + KH
    chunk_base = np.concatenate([[0], np.cumsum(Ktot)])
    NCHUNK = int(chunk_base[-1])
    KMAX = int(max(KL.max(), KH.max()))

    st_tiles = [list(range(s * tiles_per_st, min((s + 1) * tiles_per_st, TPC)))
                for s in range(NST)]

    # gather pieces: per (tile, half) split the K chunks into PIECE-sized
    # pieces.  pieces[(t,b)] = [(chunk_off, nchunks, idx_off16, gidx), ...]
    pieces = {}
    idx_off = 0
    gidx = 0
    for tt in range(TPC):
        for b, K in ((0, int(KL[tt])), (1, int(KH[tt]))):
            ps = []
            p0 = 0
            while p0 < K:
                np_ = min(PIECE, K - p0)
                ps.append((p0, np_, idx_off, gidx))
                idx_off += np_ * P // 16
                gidx += 1
                p0 += np_
            pieces[(tt, b)] = ps
    NIDX16 = idx_off
    NG = gidx

    idx_slab = np.zeros((NC, P, NIDX16), np.int16)
    dstid_slab = np.full((NC, P, NCHUNK), -1.0, np.float32)
    counts = np.zeros((NC, NG), np.int32)
    cnt = np.zeros((NC, NPAD), np.int64)

    order = np.lexsort((tb, t, c))
    co, to, tbo = c[order], t[order], tb[order]
    rowo, dido, do_ = row[order], did[order], d[order]
    np.add.at(cnt, (co, do_), 1)

    key = (co * TPC + to) * 2 + tbo
    bounds = np.concatenate([[0], np.nonzero(np.diff(key))[0] + 1, [len(key)]])
    # per (core, tile, half): row values, -1 padded to K*P
    vals = {}
    for bi in range(len(bounds) - 1):
        lo_, hi_ = int(bounds[bi]), int(bounds[bi + 1])
        if lo_ == hi_:
            continue
        cc, tt, bb = int(co[lo_]), int(to[lo_]), int(tbo[lo_])
        n = hi_ - lo_
        K = int(KL[tt]) if bb == 0 else int(KH[tt])
        v = np.full(K * P, -1, np.int16)
        v[:n] = rowo[lo_:hi_]
        vals[(cc, tt, bb)] = (v, n)
        ch0 = int(chunk_base[tt]) + (0 if bb == 0 else int(KL[tt]))
        local = np.arange(n)
        dstid_slab[cc, local % P, ch0 + local // P] = dido[lo_:hi_]

    for cc in range(NC):
        for tt in range(TPC):
            for b in (0, 1):
                K = int(KL[tt]) if b == 0 else int(KH[tt])
                v, n = vals.get((cc, tt, b), (np.full(K * P, -1, np.int16), 0))
                for (p0, np_, o16, g) in pieces[(tt, b)]:
                    pv = v[p0 * P:(p0 + np_) * P].copy()
                    nvalid = int(max(0, min(n - p0 * P, np_ * P)))
                    if nvalid == 0:
                        pv[0] = 0        # dummy valid idx (dstid -1 -> S col 0)
                        nvalid = 1
                    counts[cc, g] = nvalid
                    w = pv.reshape(np_ * P // 16, 16).T
                    idx_slab[cc, :, o16:o16 + np_ * P // 16] = np.tile(w, (8, 1))

    inv = (1.0 / np.maximum(cnt, 1)).astype(np.float32)
    shift_ok = bool(cnt[:, :npc].min() >= 1)

    return dict(
        npc=npc, half=half, TPC=TPC, NPAD=NPAD, NST=NST, st_tiles=st_tiles,
        KL=[int(v) for v in KL], KH=[int(v) for v in KH], KMAX=KMAX,
        chunk_base=[int(v) for v in chunk_base], NCHUNK=NCHUNK,
        pieces=pieces, NIDX16=NIDX16, NG=NG,
        idx_slab=idx_slab, dstid_slab=dstid_slab, counts=counts, inv=inv,
        shift_ok=shift_ok,
    )


# ---------------------------------------------------------------------------
# device kernel builder
# ---------------------------------------------------------------------------

def build_kernel(meta, in_dim, NC):
    npc, half = meta["npc"], meta["half"]
    TPC, NPAD, NST = meta["TPC"], meta["NPAD"], meta["NST"]
    NCHUNK, NIDX16, NG = meta["NCHUNK"], meta["NIDX16"], meta["NG"]
    KL, KH, KMAX = meta["KL"], meta["KH"], meta["KMAX"]
    chunk_base = meta["chunk_base"]
    pieces = meta["pieces"]
    shift = meta["shift_ok"]
    GC = (in_dim + P - 1) // P
    GPAD = GC * P
    WMAX = max(len(ts) for ts in meta["st_tiles"]) * P
    KT2MAX = max(KL[t] + KH[t] for t in range(TPC))
    # lo-gather buffer count: ~40KB/partition budget
    NBUF_LO = max(NBUF_HI + 2, min(16, (40 * 1024) // (KMAX * P * 2)))
    gq = [0]  # gather queue round-robin over all 4 SWDGE queues

    nc = bacc.Bacc("TRN2", target_bir_lowering=False, debug=False,
                   enable_asserts=False, num_devices=NC,
                   dynamic_dma_scratch_size=SCRATCH, num_swdge_queues=4)

    x_d = nc.dram_tensor("x_pad", [GPAD, NPAD], BF16, kind="ExternalInput").ap()
    w0l_d = nc.dram_tensor("W0lT", [GPAD, P], BF16, kind="ExternalInput").ap()
    w0r_d = nc.dram_tensor("W0rT", [GPAD, P], BF16, kind="ExternalInput").ap()
    w1l_d = nc.dram_tensor("W1lT", [P, P], BF16, kind="ExternalInput").ap()
    w1r_d = nc.dram_tensor("W1rT", [P, P], BF16, kind="ExternalInput").ap()
    b0_d = nc.dram_tensor("b0col", [P, 1], F32, kind="ExternalInput").ap()
    b1_d = nc.dram_tensor("b1col", [P, 1], F32, kind="ExternalInput").ap()
    inv_d = nc.dram_tensor("invt", [P, NPAD], F32, kind="ExternalInput").ap()
    idx_d = nc.dram_tensor("idx16", [P, NIDX16], I16, kind="ExternalInput").ap()
    cnts_d = nc.dram_tensor("gcnts", [P, NG], I32, kind="ExternalInput").ap()
    iota_d = nc.dram_tensor("iota", [P, P], BF16, kind="ExternalInput").ap()
    ident_d = nc.dram_tensor("ident", [P, P], BF16, kind="ExternalInput").ap()
    dst_d = nc.dram_tensor("dstid", [P, NCHUNK], BF16, kind="ExternalInput").ap()
    out_d = nc.dram_tensor("outT", [P, NPAD], BF16, kind="ExternalOutput").ap()

    with tile.TileContext(nc, num_cores=NC) as tc:
        with (
            tc.tile_pool(name="const", bufs=1) as cpool,
            tc.tile_pool(name="slab", bufs=1) as slab,
            tc.tile_pool(name="zp", bufs=4) as zpool,
            tc.tile_pool(name="ep", bufs=2) as epool,
            tc.tile_pool(name="sp", bufs=4) as spool,
            tc.tile_pool(name="ip", bufs=3) as ipool,
            tc.tile_pool(name="pz", bufs=3, space="PSUM") as pz,
            tc.tile_pool(name="pr", bufs=2, space="PSUM") as pr,
            tc.tile_pool(name="dram", bufs=1, space="DRAM") as dram,
        ):
            # ---- constants ----
            w0l_sb = cpool.tile([P, GC * P], BF16)
            w0r_sb = cpool.tile([P, GC * P], BF16)
            for gc in range(GC):
                nc.sync.dma_start(out=w0l_sb[:, gc * P:(gc + 1) * P],
                                  in_=w0l_d[gc * P:(gc + 1) * P, :])
                nc.sync.dma_start(out=w0r_sb[:, gc * P:(gc + 1) * P],
                                  in_=w0r_d[gc * P:(gc + 1) * P, :])
            w1l_sb = cpool.tile([P, P], BF16)
            nc.sync.dma_start(out=w1l_sb[:], in_=w1l_d[:])
            w1r_sb = cpool.tile([P, P], BF16)
            nc.sync.dma_start(out=w1r_sb[:], in_=w1r_d[:])
            b0_sb = cpool.tile([P, 1], F32)
            nc.sync.dma_start(out=b0_sb[:], in_=b0_d[:])
            b1_sb = cpool.tile([P, 1], F32)
            nc.sync.dma_start(out=b1_sb[:], in_=b1_d[:])
            zero_sb = cpool.tile([P, 1], BF16)
            nc.vector.memset(zero_sb[:], 0.0)
            mone_sb = cpool.tile([P, 1], BF16)
            nc.vector.memset(mone_sb[:], -1.0)
            iota_sb = cpool.tile([P, P], BF16)
            nc.sync.dma_start(out=iota_sb[:], in_=iota_d[:])
            ident_sb = cpool.tile([P, P], BF16)
            nc.sync.dma_start(out=ident_sb[:], in_=ident_d[:])
            dst_sb = cpool.tile([P, NCHUNK], BF16)
            nc.sync.dma_start(out=dst_sb[:], in_=dst_d[:])
            idx_sb = cpool.tile([P, NIDX16], I16)
            nc.sync.dma_start(out=idx_sb[:], in_=idx_d[:])
            cnts_sb = cpool.tile([P, NG], I32)
            nc.sync.dma_start(out=cnts_sb[:], in_=cnts_d[:])

            rb0_sb = slab.tile([P, NPAD], BF16)
            rb1_sb = slab.tile([P, NPAD], BF16)

            # gather slabs (manual buffer cycling); zeroed once during phase A
            # so skipped padding slots never feed NaN/Inf garbage into the PE
            glo_sb = cpool.tile([P, NBUF_LO * KMAX * P], BF16)
            ghi_sb = cpool.tile([P, NBUF_HI * KMAX * P], BF16)
            nc.vector.memset(glo_sb[:], 0.0)
            nc.vector.memset(ghi_sb[:], 0.0)

            greg = nc.gpsimd.alloc_register("gcnt")

            # ---- collective buffers ----
            def cc_pair(nm):
                i_lo = dram.tile([half, P], BF16, name=f"cci_lo{nm}")
                i_hi = dram.tile([half, P], BF16, name=f"cci_hi{nm}")
                o_lo = dram.tile([NC * half, P], BF16, addr_space="Shared",
                                 name=f"cco_lo{nm}")
                o_hi = dram.tile([NC * half, P], BF16, addr_space="Shared",
                                 name=f"cco_hi{nm}")
                return i_lo, i_hi, o_lo, o_hi

            cc0 = cc_pair("0")
            cc1 = cc_pair("1")
            rg = [list(range(NC))]

            # z/out result writes go on the Activation engine's HWDGE stream;
            # Sync stays a pure prefetch FIFO (x, inv loads).
            def z_to_cc(z_sb, tt, cc):
                r0, r1 = tt * P, min(tt * P + P, npc)
                for lo_s, hi_s, tgt, base in (
                        (r0, min(r1, half), cc[0], 0),
                        (max(r0, half), r1, cc[1], half)):
                    if hi_s > lo_s:
                        nc.scalar.dma_start(
                            out=tgt[lo_s - base:hi_s - base, :],
                            in_=z_sb[lo_s - r0:hi_s - r0, :])

            def ag(cc, which):
                nc.gpsimd.collective_compute(
                    "AllGather", ALU.bypass, replica_groups=rg,
                    ins=[cc[which][:].opt()], outs=[cc[which + 2][:].opt()])

            # ---- phase A: z0 feature-major + PE transpose; rb0 slab ----
            # 2-tile groups keep the x tiles small (xp pool is phase-A only)
            pa_groups = [list(range(s * 2, min(s * 2 + 2, TPC)))
                         for s in range((TPC + 1) // 2)]
            s_ag_a = next(i for i, g in enumerate(pa_groups)
                          if (g[-1] + 1) * P >= half)
            with (
                tc.tile_pool(name="xp", bufs=2) as xpool,
                tc.tile_pool(name="pf", bufs=2, space="PSUM") as pf,
            ):
                for s, ts in enumerate(pa_groups):
                    w = len(ts) * P
                    c0 = ts[0] * P
                    xg = xpool.tile([P, GC * w], BF16, tag="xg",
                                    padded_shape=[P, GC * 2 * P])
                    nc.sync.dma_start(
                        out=xg[:].rearrange("p (gc j) -> p gc j", gc=GC),
                        in_=x_d[:, c0:c0 + w].rearrange("(gc p) j -> p gc j", p=P))
                    r0ps = pr.tile([P, w], F32, tag="rps", padded_shape=[P, WMAX])
                    for gc in range(GC):
                        nc.tensor.matmul(out=r0ps[:],
                                         lhsT=w0r_sb[:, gc * P:(gc + 1) * P],
                                         rhs=xg[:, gc * w:(gc + 1) * w],
                                         start=(gc == 0), stop=(gc == GC - 1))
                    nc.vector.tensor_tensor(out=rb0_sb[:, c0:c0 + w], in0=r0ps[:],
                                            in1=b0_sb[:, :1].to_broadcast([P, w]),
                                            op=ALU.add)
                    zfm = pf.tile([P, w], F32, tag="zfm", padded_shape=[P, 2 * P])
                    for gc in range(GC):
                        nc.tensor.matmul(out=zfm[:],
                                         lhsT=w0l_sb[:, gc * P:(gc + 1) * P],
                                         rhs=xg[:, gc * w:(gc + 1) * w],
                                         start=(gc == 0), stop=(gc == GC - 1))
                    zfm_sb = zpool.tile([P, w], BF16, tag="zfm_sb",
                                        padded_shape=[P, 2 * P])
                    nc.vector.tensor_copy(out=zfm_sb[:], in_=zfm[:])
                    for ti, tt in enumerate(ts):
                        zT = pz.tile([P, P], F32, tag="zps")
                        nc.tensor.matmul(out=zT[:],
                                         lhsT=zfm_sb[:, ti * P:(ti + 1) * P],
                                         rhs=ident_sb[:], start=True, stop=True)
                        z0sb = zpool.tile([P, P], BF16, tag="zsb")
                        nc.vector.tensor_copy(out=z0sb[:], in_=zT[:])
                        z_to_cc(z0sb, tt, cc0)
                    if s == s_ag_a:
                        ag(cc0, 0)
            ag(cc0, 1)

            # pa opens after phase A's pf pool closes (PSUM is 8 banks; the
            # two phases each use 7)
            pa_cm = tc.tile_pool(name="pa", bufs=2, space="PSUM")
            pa = pa_cm.__enter__()

            # ---- aggregation machinery -------------------------------------
            def issue_gather(table_pair, slab_sb, nbuf, seq, b):
                """Issue all pieces of the (tile-half) gather stream element
                `seq` = (layer, tile); buffer = global issue counter % nbuf."""
                layer, tt = seq
                buf = (layer * TPC + tt) % nbuf
                base = buf * KMAX * P
                table = table_pair[layer][b + 2]
                for (p0, np_, o16, g) in pieces[(tt, b)]:
                    gq[0] = (gq[0] + 1) % 4
                    nc.gpsimd.reg_load(greg, cnts_sb[0:1, g:g + 1])
                    nc.gpsimd.dma_gather(
                        out_ap=slab_sb[:, base + p0 * P:base + (p0 + np_) * P]
                        .rearrange("p (k e) -> p k e", e=P),
                        in_ap=table[:],
                        idxs_ap=idx_sb[:, o16:o16 + np_ * 8],
                        num_idxs=np_ * P, num_idxs_reg=greg, elem_size=P,
                        single_packet=True, queue_num=gq[0])

            tables = (cc0, cc1)
            lo_seq = [(l, t) for l in (0, 1) for t in range(TPC)]
            hi_seq = list(lo_seq)
            state = {"lo": 0, "hi": 0}

            def pump_lo(limit):
                while state["lo"] < min(limit, 2 * TPC):
                    issue_gather(tables, glo_sb, NBUF_LO, lo_seq[state["lo"]], 0)
                    state["lo"] += 1

            def pump_hi(limit):
                while state["hi"] < min(limit, 2 * TPC):
                    issue_gather(tables, ghi_sb, NBUF_HI, hi_seq[state["hi"]], 1)
                    state["hi"] += 1

            def aggregate_tile(layer, tt, aggps, ti):
                """S build + chunk matmuls for one tile into group psum col ti."""
                nch = KL[tt] + KH[tt]
                cb0 = chunk_base[tt]
                s_sb = spool.tile([P, nch * P], BF16, tag="ssb",
                                  padded_shape=[P, KT2MAX * P])
                nc.vector.tensor_tensor(
                    out=s_sb[:].rearrange("p (n e) -> p n e", e=P),
                    in0=dst_sb[:, cb0:cb0 + nch].unsqueeze(2)
                    .to_broadcast([P, nch, P]),
                    in1=iota_sb[:].unsqueeze(1).to_broadcast([P, nch, P]),
                    op=ALU.is_equal)
                lobuf = ((layer * TPC + tt) % NBUF_LO) * KMAX * P
                hibuf = ((layer * TPC + tt) % NBUF_HI) * KMAX * P
                for j in range(nch):
                    if j < KL[tt]:
                        g_ap = glo_sb[:, lobuf + j * P:lobuf + (j + 1) * P]
                    else:
                        jj = j - KL[tt]
                        g_ap = ghi_sb[:, hibuf + jj * P:hibuf + (jj + 1) * P]
                    nc.tensor.matmul(out=aggps[:, ti * P:(ti + 1) * P],
                                     lhsT=g_ap,
                                     rhs=s_sb[:, j * P:(j + 1) * P],
                                     start=(j == 0), stop=(j == nch - 1))

            s_ag = ((half + P - 1) // P - 1) // len(meta["st_tiles"][0])

            def run_layer(layer, rb_slab, out_cb, mid_cb=None):
                for s, ts in enumerate(meta["st_tiles"]):
                    w = len(ts) * P
                    c0 = ts[0] * P
                    aggps = pa.tile([P, w], F32, tag="aggps",
                                    padded_shape=[P, WMAX])
                    for ti, tt in enumerate(ts):
                        step = layer * TPC + tt
                        hi_limit = step + HI_LOOK + 1
                        lo_limit = step + NBUF_LO
                        if layer == 0:
                            # layer-1 hi gathers would deadlock: their AG1-hi
                            # trigger is issued after this loop.  layer-1 lo
                            # gathers gate on AG1-lo (triggered mid-layer at
                            # group s_ag); hold them until L1_GATE so they
                            # don't head-of-line block layer-0 hi gathers.
                            hi_limit = min(hi_limit, TPC)
                            if tt < L1_GATE:
                                lo_limit = min(lo_limit, TPC)
                        pump_hi(hi_limit)
                        pump_lo(lo_limit)
                        aggregate_tile(layer, tt, aggps, ti)
                    invt = ipool.tile([P, w], F32, tag="invt",
                                      padded_shape=[P, WMAX])
                    nc.sync.dma_start(out=invt[:], in_=inv_d[:, c0:c0 + w])
                    x2 = epool.tile([P, w], BF16, tag="x2", padded_shape=[P, WMAX])
                    nc.vector.tensor_tensor(out=x2[:], in0=aggps[:],
                                            in1=invt[:], op=ALU.mult)
                    x3 = epool.tile([P, w], BF16, tag="x3", padded_shape=[P, WMAX])
                    nc.vector.tensor_tensor(out=x3[:], in0=x2[:],
                                            in1=rb_slab[:, c0:c0 + w], op=ALU.add)
                    xm = epool.tile([P, w], BF16, tag="xm", padded_shape=[P, WMAX])
                    nc.scalar.activation(out=xm[:], in_=x3[:], func=AF.Relu)
                    xc = epool.tile([P, w], BF16, tag="xc", padded_shape=[P, WMAX])
                    nc.vector.tensor_tensor(out=xc[:], in0=x3[:],
                                            in1=zero_sb[:, :1].to_broadcast([P, w]),
                                            op=ALU.min)
                    xe = epool.tile([P, w], BF16, tag="xe", padded_shape=[P, WMAX])
                    nc.scalar.activation(out=xe[:], in_=xc[:], func=AF.Exp)
                    h = epool.tile([P, w], BF16, tag="h", padded_shape=[P, WMAX])
                    nc.vector.tensor_tensor(out=h[:], in0=xm[:], in1=xe[:],
                                            op=ALU.add)
                    if not shift:
                        h2 = epool.tile([P, w], BF16, tag="h2",
                                        padded_shape=[P, WMAX])
                        nc.vector.tensor_tensor(
                            out=h2[:], in0=h[:],
                            in1=mone_sb[:, :1].to_broadcast([P, w]), op=ALU.add)
                        h = h2
                    out_cb(s, ts, w, c0, h)
                    if mid_cb is not None and s == s_ag:
                        mid_cb()

            # ---- layer 0 aggregate -> h1T -> z1/rb1T ----
            def l0_out(s, ts, w, c0, h):
                for ti, tt in enumerate(ts):
                    z1ps = pz.tile([P, P], F32, tag="zps")
                    nc.tensor.matmul(out=z1ps[:],
                                     lhsT=h[:, ti * P:(ti + 1) * P],
                                     rhs=w1l_sb[:], start=True, stop=True)
                    z1sb = zpool.tile([P, P], BF16, tag="zsb")
                    nc.vector.tensor_copy(out=z1sb[:], in_=z1ps[:])
                    z_to_cc(z1sb, tt, cc1)
                r1ps = pr.tile([P, w], F32, tag="rps", padded_shape=[P, WMAX])
                nc.tensor.matmul(out=r1ps[:], lhsT=w1r_sb[:], rhs=h[:],
                                 start=True, stop=True)
                nc.vector.tensor_tensor(out=rb1_sb[:, c0:c0 + w], in0=r1ps[:],
                                        in1=b1_sb[:, :1].to_broadcast([P, w]),
                                        op=ALU.add)

            # prologue: fill lo lookahead, then a short hi lead
            pump_lo(NBUF_LO)
            pump_hi(HI_LOOK)

            run_layer(0, rb0_sb, l0_out, mid_cb=lambda: ag(cc1, 0))
            ag(cc1, 1)

            # ---- layer 1 aggregate -> output ----
            def l1_out(s, ts, w, c0, h):
                nc.scalar.dma_start(out=out_d[:, c0:c0 + w], in_=h[:])

            run_layer(1, rb1_sb, l1_out)
            pa_cm.__exit__(None, None, None)

    nc.compile()
    return nc


# ---------------------------------------------------------------------------
# entry point
# ---------------------------------------------------------------------------

def _bf16(a):
    return np.asarray(a, np.float32).astype(ml_dtypes.bfloat16)


def _prepare(x, knn_edge_index, W_l0, b_l0, W_r0, W_l1, b_l1, W_r1,
             NC=8, tiles_per_st=4):
    x = np.asarray(x, np.float32)
    e = np.asarray(knn_edge_index)
    in_dim, N = x.shape
    src, dst = e[0].astype(np.int64), e[1].astype(np.int64)
    meta = build_meta(N, NC, dst, src, tiles_per_st)
    npc, NPAD = meta["npc"], meta["NPAD"]
    GC = (in_dim + P - 1) // P
    GPAD = GC * P

    W_l1 = np.asarray(W_l1, np.float32)
    W_r1 = np.asarray(W_r1, np.float32)
    b1 = np.asarray(b_l1, np.float32)
    if meta["shift_ok"]:
        b1 = b1 - W_l1.sum(axis=1) - W_r1.sum(axis=1)

    w0l = np.zeros((GPAD, P), np.float32); w0l[:in_dim] = np.asarray(W_l0).T
    w0r = np.zeros((GPAD, P), np.float32); w0r[:in_dim] = np.asarray(W_r0).T
    shared = {
        "W0lT": _bf16(w0l), "W0rT": _bf16(w0r),
        "W1lT": _bf16(np.ascontiguousarray(W_l1.T)),
        "W1rT": _bf16(np.ascontiguousarray(W_r1.T)),
        "b0col": np.asarray(b_l0, np.float32).reshape(P, 1),
        "b1col": b1.reshape(P, 1),
        "iota": _bf16(np.broadcast_to(np.arange(P, dtype=np.float32), (P, P))),
        "ident": _bf16(np.eye(P, dtype=np.float32)),
    }
    in_maps = []
    for c in range(NC):
        xp = np.zeros((GPAD, NPAD), np.float32)
        xp[:in_dim, :npc] = x[:, c * npc:(c + 1) * npc]
        m = dict(shared)
        m["x_pad"] = _bf16(xp)
        m["invt"] = np.ascontiguousarray(
            np.broadcast_to(meta["inv"][c], (P, NPAD)))
        m["idx16"] = np.ascontiguousarray(meta["idx_slab"][c])
        m["gcnts"] = np.ascontiguousarray(
            np.broadcast_to(meta["counts"][c], (P, meta["NG"])).astype(np.int32))
        m["dstid"] = _bf16(meta["dstid_slab"][c])
        in_maps.append(m)
    return meta, in_dim, in_maps


def run(inputs, NC=8, tiles_per_st=4, trace=False, **run_kwargs):
    meta, in_dim, in_maps = _prepare(**inputs, NC=NC, tiles_per_st=tiles_per_st)
    nc = build_kernel(meta, in_dim, NC)
    res = bass_utils.run_bass_kernel_spmd(
        nc, in_maps, core_ids=list(range(NC)), trace=trace, **run_kwargs)
    npc = meta["npc"]
    sub = 1.0 if meta["shift_ok"] else 0.0
    out = np.concatenate(
        [res.results[c]["outT"][:, :npc].T.astype(np.float32) - sub
         for c in range(NC)], axis=0)
    return np.ascontiguousarray(out), res


def kernel(**inputs) -> np.ndarray:
    out, _ = run(inputs)
    return out


# revision 12
# speedup vs baseline: 1.1800x; 1.1693x over previous
"""Trainium2 Bass kernel for nn_CellEncoder (2-layer GraphSAGE, mean aggregation).

Strategy (8 NeuronCores, SPMD, node-partitioned), v3:
  - Core c owns nodes [c*npc, (c+1)*npc).  Aggregation is linear, so the
    dense transform is applied FIRST: z = h @ W_l.T reduces gather width
    from in_dim (1000) to emb (128) values per edge.  All tables/operands
    are bf16 (PSUM accumulation fp32); tolerance is 2e-2, bf16 ~5e-3.
  - Per layer: each core computes z for its own nodes, contributes two
    half-slabs to two AllGathers forming table_lo/table_hi (rows < 32768
    so int16 dma_gather indices address them at 256B stride).
  - Edges grouped by (dst tile, half); slots packed into 128-slot chunks.
    dma_gather pulls slot rows into SBUF on 4 SWDGE queues round-robin.
    v3: gathers are per-(tile,half) pieces of <= 8 chunks with padding
    slots as NEGATIVE indices at the gather tail -- the DMA skips them --
    and num_idxs_reg is reg_load-ed from a per-core count table, so the
    descriptor stream is exactly the core's real edge count (v2 gathered
    ~12% padding descriptors: max-over-core chunk rounding).
  - The int16 index slab for a layer (both layers share it -- same edges)
    is loaded ONCE into SBUF during phase A; the gather stream's only
    dependencies are the z-table AllGathers and gather-buffer reuse.
    Gather buffers are a fixed SBUF slab with manual cycling (NBUF_LO
    lo-tiles in flight, NBUF_HI hi-tiles); lo gathers run ahead to cover
    the AG-hi window, and layer-1 lo gathers are issued during layer-0's
    tail tiles once AG1-lo has completed.
  - One-hot scatter matrices S[e,d] = (dst(e)==d) built on DVE per tile
    with a single batched is_equal (stride-0 broadcast APs).  PE
    accumulates aggT[f,d] += G_chunk.T @ S_chunk in PSUM per group of 4
    tiles.  Skipped padding slots hold zeros/stale z rows (finite) and
    S = 0 there, so they contribute nothing.
  - Phase A computes z0 feature-major with stationary W_l0 (8 LDWEIGHTS
    per 2-tile group instead of 8 per tile), then transposes each tile to
    node-major on the PE against an identity matrix.
  - ELU's "-1" is folded out: the device computes h~ = elu(x)+1
    (= max(x,0)+exp(min(x,0))); the next layer's bias is adjusted on the
    host (b1' = b1 - W_l1@1 - W_r1@1) and the host subtracts 1 from the
    final output.  (requires min in-degree >= 1, checked on host).
  - Epilogue feature-major; Relu/Exp on the scalar engine, rest on DVE.
    Output written bf16 [128, NPAD]; host casts, subtracts 1, transposes,
    trims.

kernel(**inputs) takes FULL inputs, shards internally, runs one NEFF on
cores 0-7 via bass_utils.run_bass_kernel_spmd, returns the full output.
"""
import os
import sys

import numpy as np

for _p in ("/opt/trn_rl_repo", "/root/.axon_site/_ro/trn_rl_repo"):
    if os.path.isdir(_p) and _p not in sys.path:
        sys.path.append(_p)

import ml_dtypes

import concourse.bass as bass
import concourse.bacc as bacc
import concourse.mybir as mybir
import concourse.tile as tile
from concourse import bass_utils

P = 128
F32 = mybir.dt.float32
BF16 = mybir.dt.bfloat16
I16 = mybir.dt.int16
I32 = mybir.dt.int32
AF = mybir.ActivationFunctionType
ALU = mybir.AluOpType

# SWDGE descriptor-ring sizing: ring holds scratch//16 descriptors per queue;
# <=1024-idx gathers take the single_packet fast path and 3 fit in a ring.
SCRATCH = 49152
PIECE = 8          # chunks per gather piece (1024 idxs)
NBUF_HI = 6        # hi-gather tile buffers in flight
HI_LOOK = 4        # hi gather issue lead (tiles); must be < NBUF_HI
L1_GATE = 40       # layer-0 tile index from which layer-1 lo gathers issue


def build_meta(N, NC, dst, src, tiles_per_st):
    """Static chunk structure (shared across cores; max-over-core sizes),
    per-core gather-index / dst-id slabs with -1 padding tails, and per-core
    per-gather-piece valid-index counts."""
    npc = N // NC
    half = npc // 2
    TPC = (npc + P - 1) // P
    NPAD = TPC * P
    NST = (TPC + tiles_per_st - 1) // tiles_per_st

    c = dst // npc
    d = (dst - c * npc).astype(np.int64)
    t = d // P
    did = d % P
    sc = src // npc
    sp = src - sc * npc
    tb = (sp >= half).astype(np.int64)
    row = sc * half + np.where(tb == 0, sp, sp - half)
    assert row.max() < 32768

    nlohi = np.zeros((NC, TPC, 2), np.int64)
    np.add.at(nlohi, (c, t, tb), 1)
    KL = np.maximum(1, (nlohi[:, :, 0].max(axis=0) + P - 1) // P)
    KH = np.maximum(1, (nlohi[:, :, 1].max(axis=0) + P - 1) // P)

    Ktot = KL + KH
    chunk_base = np.concatenate([[0], np.cumsum(Ktot)])
    NCHUNK = int(chunk_base[-1])
    KMAX = int(max(KL.max(), KH.max()))

    st_tiles = [list(range(s * tiles_per_st, min((s + 1) * tiles_per_st, TPC)))
                for s in range(NST)]

    # gather pieces: per (tile, half) split the K chunks into PIECE-sized
    # pieces.  pieces[(t,b)] = [(chunk_off, nchunks, idx_off16, gidx), ...]
    pieces = {}
    idx_off = 0
    gidx = 0
    for tt in range(TPC):
        for b, K in ((0, int(KL[tt])), (1, int(KH[tt]))):
            ps = []
            p0 = 0
            while p0 < K:
                np_ = min(PIECE, K - p0)
                ps.append((p0, np_, idx_off, gidx))
                idx_off += np_ * P // 16
                gidx += 1
                p0 += np_
            pieces[(tt, b)] = ps
    NIDX16 = idx_off
    NG = gidx

    idx_slab = np.zeros((NC, P, NIDX16), np.int16)
    dstid_slab = np.full((NC, P, NCHUNK), -1.0, np.float32)
    counts = np.zeros((NC, NG), np.int32)
    cnt = np.zeros((NC, NPAD), np.int64)

    order = np.lexsort((tb, t, c))
    co, to, tbo = c[order], t[order], tb[order]
    rowo, dido, do_ = row[order], did[order], d[order]
    np.add.at(cnt, (co, do_), 1)

    key = (co * TPC + to) * 2 + tbo
    bounds = np.concatenate([[0], np.nonzero(np.diff(key))[0] + 1, [len(key)]])
    # per (core, tile, half): row values, -1 padded to K*P
    vals = {}
    for bi in range(len(bounds) - 1):
        lo_, hi_ = int(bounds[bi]), int(bounds[bi + 1])
        if lo_ == hi_:
            continue
        cc, tt, bb = int(co[lo_]), int(to[lo_]), int(tbo[lo_])
        n = hi_ - lo_
        K = int(KL[tt]) if bb == 0 else int(KH[tt])
        v = np.full(K * P, -1, np.int16)
        v[:n] = rowo[lo_:hi_]
        vals[(cc, tt, bb)] = (v, n)
        ch0 = int(chunk_base[tt]) + (0 if bb == 0 else int(KL[tt]))
        local = np.arange(n)
        dstid_slab[cc, local % P, ch0 + local // P] = dido[lo_:hi_]

    for cc in range(NC):
        for tt in range(TPC):
            for b in (0, 1):
                K = int(KL[tt]) if b == 0 else int(KH[tt])
                v, n = vals.get((cc, tt, b), (np.full(K * P, -1, np.int16), 0))
                for (p0, np_, o16, g) in pieces[(tt, b)]:
                    pv = v[p0 * P:(p0 + np_) * P].copy()
                    nvalid = int(max(0, min(n - p0 * P, np_ * P)))
                    if nvalid == 0:
                        pv[0] = 0        # dummy valid idx (dstid -1 -> S col 0)
                        nvalid = 1
                    counts[cc, g] = nvalid
                    w = pv.reshape(np_ * P // 16, 16).T
                    idx_slab[cc, :, o16:o16 + np_ * P // 16] = np.tile(w, (8, 1))

    inv = (1.0 / np.maximum(cnt, 1)).astype(np.float32)
    shift_ok = bool(cnt[:, :npc].min() >= 1)

    return dict(
        npc=npc, half=half, TPC=TPC, NPAD=NPAD, NST=NST, st_tiles=st_tiles,
        KL=[int(v) for v in KL], KH=[int(v) for v in KH], KMAX=KMAX,
        chunk_base=[int(v) for v in chunk_base], NCHUNK=NCHUNK,
        pieces=pieces, NIDX16=NIDX16, NG=NG,
        idx_slab=idx_slab, dstid_slab=dstid_slab, counts=counts, inv=inv,
        shift_ok=shift_ok,
    )


# ---------------------------------------------------------------------------
# device kernel builder
# ---------------------------------------------------------------------------

def build_kernel(meta, in_dim, NC):
    npc, half = meta["npc"], meta["half"]
    TPC, NPAD, NST = meta["TPC"], meta["NPAD"], meta["NST"]
    NCHUNK, NIDX16, NG = meta["NCHUNK"], meta["NIDX16"], meta["NG"]
    KL, KH, KMAX = meta["KL"], meta["KH"], meta["KMAX"]
    chunk_base = meta["chunk_base"]
    pieces = meta["pieces"]
    shift = meta["shift_ok"]
    GC = (in_dim + P - 1) // P
    GPAD = GC * P
    WMAX = max(len(ts) for ts in meta["st_tiles"]) * P
    KT2MAX = max(KL[t] + KH[t] for t in range(TPC))
    # lo-gather buffer count: ~40KB/partition budget
    NBUF_LO = max(NBUF_HI + 2, min(16, (40 * 1024) // (KMAX * P * 2)))
    # least-loaded queue assignment (by chunk count): a plain round-robin
    # aliases with the big/small piece alternation and starves half the
    # queues (measured: q1/q3 at 26-31 GB/s while q0/q2 idle)
    qload = [0, 0, 0, 0]

    nc = bacc.Bacc("TRN2", target_bir_lowering=False, debug=False,
                   enable_asserts=False, num_devices=NC,
                   dynamic_dma_scratch_size=SCRATCH, num_swdge_queues=4)

    x_d = nc.dram_tensor("x_pad", [GPAD, NPAD], BF16, kind="ExternalInput").ap()
    w0l_d = nc.dram_tensor("W0lT", [GPAD, P], BF16, kind="ExternalInput").ap()
    w0r_d = nc.dram_tensor("W0rT", [GPAD, P], BF16, kind="ExternalInput").ap()
    w1l_d = nc.dram_tensor("W1lT", [P, P], BF16, kind="ExternalInput").ap()
    w1r_d = nc.dram_tensor("W1rT", [P, P], BF16, kind="ExternalInput").ap()
    b0_d = nc.dram_tensor("b0col", [P, 1], F32, kind="ExternalInput").ap()
    b1_d = nc.dram_tensor("b1col", [P, 1], F32, kind="ExternalInput").ap()
    inv_d = nc.dram_tensor("invt", [P, NPAD], F32, kind="ExternalInput").ap()
    idx_d = nc.dram_tensor("idx16", [P, NIDX16], I16, kind="ExternalInput").ap()
    cnts_d = nc.dram_tensor("gcnts", [P, NG], I32, kind="ExternalInput").ap()
    iota_d = nc.dram_tensor("iota", [P, P], BF16, kind="ExternalInput").ap()
    ident_d = nc.dram_tensor("ident", [P, P], BF16, kind="ExternalInput").ap()
    dst_d = nc.dram_tensor("dstid", [P, NCHUNK], BF16, kind="ExternalInput").ap()
    out_d = nc.dram_tensor("outT", [P, NPAD], BF16, kind="ExternalOutput").ap()

    with tile.TileContext(nc, num_cores=NC) as tc:
        with (
            tc.tile_pool(name="const", bufs=1) as cpool,
            tc.tile_pool(name="slab", bufs=1) as slab,
            tc.tile_pool(name="zp", bufs=4) as zpool,
            tc.tile_pool(name="ep", bufs=2) as epool,
            tc.tile_pool(name="sp", bufs=4) as spool,
            tc.tile_pool(name="ip", bufs=3) as ipool,
            tc.tile_pool(name="pz", bufs=3, space="PSUM") as pz,
            tc.tile_pool(name="pr", bufs=2, space="PSUM") as pr,
            tc.tile_pool(name="dram", bufs=1, space="DRAM") as dram,
        ):
            # ---- constants ----
            w0l_sb = cpool.tile([P, GC * P], BF16)
            w0r_sb = cpool.tile([P, GC * P], BF16)
            for gc in range(GC):
                nc.sync.dma_start(out=w0l_sb[:, gc * P:(gc + 1) * P],
                                  in_=w0l_d[gc * P:(gc + 1) * P, :])
                nc.sync.dma_start(out=w0r_sb[:, gc * P:(gc + 1) * P],
                                  in_=w0r_d[gc * P:(gc + 1) * P, :])
            w1l_sb = cpool.tile([P, P], BF16)
            nc.sync.dma_start(out=w1l_sb[:], in_=w1l_d[:])
            w1r_sb = cpool.tile([P, P], BF16)
            nc.sync.dma_start(out=w1r_sb[:], in_=w1r_d[:])
            b0_sb = cpool.tile([P, 1], F32)
            nc.sync.dma_start(out=b0_sb[:], in_=b0_d[:])
            b1_sb = cpool.tile([P, 1], F32)
            nc.sync.dma_start(out=b1_sb[:], in_=b1_d[:])
            zero_sb = cpool.tile([P, 1], BF16)
            nc.vector.memset(zero_sb[:], 0.0)
            mone_sb = cpool.tile([P, 1], BF16)
            nc.vector.memset(mone_sb[:], -1.0)
            # iota materialized at full S width: a contiguous in1 operand for
            # the batched is_equal (stride-0 broadcasts run at reduced rate)
            iota_sb = cpool.tile([P, KT2MAX * P], BF16)
            for k in range(KT2MAX):
                nc.sync.dma_start(out=iota_sb[:, k * P:(k + 1) * P], in_=iota_d[:])
            ident_sb = cpool.tile([P, P], BF16)
            nc.sync.dma_start(out=ident_sb[:], in_=ident_d[:])
            dst_sb = cpool.tile([P, NCHUNK], BF16)
            nc.sync.dma_start(out=dst_sb[:], in_=dst_d[:])
            idx_sb = cpool.tile([P, NIDX16], I16)
            nc.sync.dma_start(out=idx_sb[:], in_=idx_d[:])
            cnts_sb = cpool.tile([P, NG], I32)
            nc.sync.dma_start(out=cnts_sb[:], in_=cnts_d[:])

            rb0_sb = slab.tile([P, NPAD], BF16)
            rb1_sb = slab.tile([P, NPAD], BF16)

            # gather slabs (manual buffer cycling); zeroed once during phase A
            # so skipped padding slots never feed NaN/Inf garbage into the PE
            glo_sb = cpool.tile([P, NBUF_LO * KMAX * P], BF16)
            ghi_sb = cpool.tile([P, NBUF_HI * KMAX * P], BF16)
            # on gpsimd: the Pool engine is idle through phase A, DVE is not
            nc.gpsimd.memset(glo_sb[:], 0.0)
            nc.gpsimd.memset(ghi_sb[:], 0.0)

            greg = nc.gpsimd.alloc_register("gcnt")

            # ---- collective buffers ----
            def cc_pair(nm):
                i_lo = dram.tile([half, P], BF16, name=f"cci_lo{nm}")
                i_hi = dram.tile([half, P], BF16, name=f"cci_hi{nm}")
                o_lo = dram.tile([NC * half, P], BF16, addr_space="Shared",
                                 name=f"cco_lo{nm}")
                o_hi = dram.tile([NC * half, P], BF16, addr_space="Shared",
                                 name=f"cco_hi{nm}")
                return i_lo, i_hi, o_lo, o_hi

            cc0 = cc_pair("0")
            cc1 = cc_pair("1")
            rg = [list(range(NC))]

            # z/out result writes go on the Activation engine's HWDGE stream;
            # Sync stays a pure prefetch FIFO (x, inv loads).
            def z_to_cc(z_sb, tt, cc):
                r0, r1 = tt * P, min(tt * P + P, npc)
                for lo_s, hi_s, tgt, base in (
                        (r0, min(r1, half), cc[0], 0),
                        (max(r0, half), r1, cc[1], half)):
                    if hi_s > lo_s:
                        nc.scalar.dma_start(
                            out=tgt[lo_s - base:hi_s - base, :],
                            in_=z_sb[lo_s - r0:hi_s - r0, :])

            def ag(cc, which):
                nc.gpsimd.collective_compute(
                    "AllGather", ALU.bypass, replica_groups=rg,
                    ins=[cc[which][:].opt()], outs=[cc[which + 2][:].opt()])

            # ---- phase A: z0 feature-major + PE transpose; rb0 slab ----
            # 2-tile groups keep the x tiles small (xp pool is phase-A only)
            pa_groups = [list(range(s * 2, min(s * 2 + 2, TPC)))
                         for s in range((TPC + 1) // 2)]
            s_ag_a = next(i for i, g in enumerate(pa_groups)
                          if (g[-1] + 1) * P >= half)
            with (
                tc.tile_pool(name="xp", bufs=2) as xpool,
                tc.tile_pool(name="pf", bufs=2, space="PSUM") as pf,
            ):
                for s, ts in enumerate(pa_groups):
                    w = len(ts) * P
                    c0 = ts[0] * P
                    xg = xpool.tile([P, GC * w], BF16, tag="xg",
                                    padded_shape=[P, GC * 2 * P])
                    nc.sync.dma_start(
                        out=xg[:].rearrange("p (gc j) -> p gc j", gc=GC),
                        in_=x_d[:, c0:c0 + w].rearrange("(gc p) j -> p gc j", p=P))
                    r0ps = pr.tile([P, w], F32, tag="rps", padded_shape=[P, WMAX])
                    for gc in range(GC):
                        nc.tensor.matmul(out=r0ps[:],
                                         lhsT=w0r_sb[:, gc * P:(gc + 1) * P],
                                         rhs=xg[:, gc * w:(gc + 1) * w],
                                         start=(gc == 0), stop=(gc == GC - 1))
                    nc.vector.tensor_tensor(out=rb0_sb[:, c0:c0 + w], in0=r0ps[:],
                                            in1=b0_sb[:, :1].to_broadcast([P, w]),
                                            op=ALU.add)
                    zfm = pf.tile([P, w], F32, tag="zfm", padded_shape=[P, 2 * P])
                    for gc in range(GC):
                        nc.tensor.matmul(out=zfm[:],
                                         lhsT=w0l_sb[:, gc * P:(gc + 1) * P],
                                         rhs=xg[:, gc * w:(gc + 1) * w],
                                         start=(gc == 0), stop=(gc == GC - 1))
                    zfm_sb = zpool.tile([P, w], BF16, tag="zfm_sb",
                                        padded_shape=[P, 2 * P])
                    nc.vector.tensor_copy(out=zfm_sb[:], in_=zfm[:])
                    for ti, tt in enumerate(ts):
                        zT = pz.tile([P, P], F32, tag="zps")
                        nc.tensor.matmul(out=zT[:],
                                         lhsT=zfm_sb[:, ti * P:(ti + 1) * P],
                                         rhs=ident_sb[:], start=True, stop=True)
                        z0sb = zpool.tile([P, P], BF16, tag="zsb")
                        nc.vector.tensor_copy(out=z0sb[:], in_=zT[:])
                        z_to_cc(z0sb, tt, cc0)
                    if s == s_ag_a:
                        ag(cc0, 0)
            ag(cc0, 1)

            # pa opens after phase A's pf pool closes (PSUM is 8 banks; the
            # two phases each use 7)
            pa_cm = tc.tile_pool(name="pa", bufs=2, space="PSUM")
            pa = pa_cm.__enter__()

            # ---- aggregation machinery -------------------------------------
            def issue_gather(table_pair, slab_sb, nbuf, seq, b):
                """Issue all pieces of the (tile-half) gather stream element
                `seq` = (layer, tile); buffer = global issue counter % nbuf."""
                layer, tt = seq
                buf = (layer * TPC + tt) % nbuf
                base = buf * KMAX * P
                table = table_pair[layer][b + 2]
                for (p0, np_, o16, g) in pieces[(tt, b)]:
                    q = min(range(4), key=lambda i: qload[i])
                    qload[q] += np_
                    nc.gpsimd.reg_load(greg, cnts_sb[0:1, g:g + 1])
                    nc.gpsimd.dma_gather(
                        out_ap=slab_sb[:, base + p0 * P:base + (p0 + np_) * P]
                        .rearrange("p (k e) -> p k e", e=P),
                        in_ap=table[:],
                        idxs_ap=idx_sb[:, o16:o16 + np_ * 8],
                        num_idxs=np_ * P, num_idxs_reg=greg, elem_size=P,
                        single_packet=True, queue_num=q)

            tables = (cc0, cc1)
            lo_seq = [(l, t) for l in (0, 1) for t in range(TPC)]
            hi_seq = list(lo_seq)
            state = {"lo": 0, "hi": 0}

            def pump_lo(limit):
                while state["lo"] < min(limit, 2 * TPC):
                    issue_gather(tables, glo_sb, NBUF_LO, lo_seq[state["lo"]], 0)
                    state["lo"] += 1

            def pump_hi(limit):
                while state["hi"] < min(limit, 2 * TPC):
                    issue_gather(tables, ghi_sb, NBUF_HI, hi_seq[state["hi"]], 1)
                    state["hi"] += 1

            def aggregate_tile(layer, tt, aggps, ti):
                """S build + chunk matmuls for one tile into group psum col ti."""
                nch = KL[tt] + KH[tt]
                cb0 = chunk_base[tt]
                s_sb = spool.tile([P, nch * P], BF16, tag="ssb",
                                  padded_shape=[P, KT2MAX * P])
                nc.vector.tensor_tensor(
                    out=s_sb[:].rearrange("p (n e) -> p n e", e=P),
                    in0=dst_sb[:, cb0:cb0 + nch].unsqueeze(2)
                    .to_broadcast([P, nch, P]),
                    in1=iota_sb[:, :nch * P].rearrange("p (n e) -> p n e", e=P),
                    op=ALU.is_equal)
                lobuf = ((layer * TPC + tt) % NBUF_LO) * KMAX * P
                hibuf = ((layer * TPC + tt) % NBUF_HI) * KMAX * P
                for j in range(nch):
                    if j < KL[tt]:
                        g_ap = glo_sb[:, lobuf + j * P:lobuf + (j + 1) * P]
                    else:
                        jj = j - KL[tt]
                        g_ap = ghi_sb[:, hibuf + jj * P:hibuf + (jj + 1) * P]
                    nc.tensor.matmul(out=aggps[:, ti * P:(ti + 1) * P],
                                     lhsT=g_ap,
                                     rhs=s_sb[:, j * P:(j + 1) * P],
                                     start=(j == 0), stop=(j == nch - 1))

            s_ag = ((half + P - 1) // P - 1) // len(meta["st_tiles"][0])

            def run_layer(layer, rb_slab, out_cb, mid_cb=None):
                for s, ts in enumerate(meta["st_tiles"]):
                    w = len(ts) * P
                    c0 = ts[0] * P
                    aggps = pa.tile([P, w], F32, tag="aggps",
                                    padded_shape=[P, WMAX])
                    for ti, tt in enumerate(ts):
                        step = layer * TPC + tt
                        hi_limit = step + HI_LOOK + 1
                        lo_limit = step + NBUF_LO
                        if layer == 0:
                            # layer-1 hi gathers would deadlock: their AG1-hi
                            # trigger is issued after this loop.  layer-1 lo
                            # gathers gate on AG1-lo (triggered mid-layer at
                            # group s_ag); hold them until L1_GATE so they
                            # don't head-of-line block layer-0 hi gathers.
                            hi_limit = min(hi_limit, TPC)
                            if tt < L1_GATE:
                                lo_limit = min(lo_limit, TPC)
                        pump_hi(hi_limit)
                        pump_lo(lo_limit)
                        aggregate_tile(layer, tt, aggps, ti)
                    invt = ipool.tile([P, w], F32, tag="invt",
                                      padded_shape=[P, WMAX])
                    nc.sync.dma_start(out=invt[:], in_=inv_d[:, c0:c0 + w])
                    x2 = epool.tile([P, w], BF16, tag="x2", padded_shape=[P, WMAX])
                    nc.vector.tensor_tensor(out=x2[:], in0=aggps[:],
                                            in1=invt[:], op=ALU.mult)
                    x3 = epool.tile([P, w], BF16, tag="x3", padded_shape=[P, WMAX])
                    nc.vector.tensor_tensor(out=x3[:], in0=x2[:],
                                            in1=rb_slab[:, c0:c0 + w], op=ALU.add)
                    xm = epool.tile([P, w], BF16, tag="xm", padded_shape=[P, WMAX])
                    nc.scalar.activation(out=xm[:], in_=x3[:], func=AF.Relu)
                    # exp(min(x,0)) = exp(-relu(-x)): both on the scalar
                    # engine via the activation scale, freeing a DVE op
                    xc = epool.tile([P, w], BF16, tag="xc", padded_shape=[P, WMAX])
                    nc.scalar.activation(out=xc[:], in_=x3[:], func=AF.Relu,
                                         scale=-1.0)
                    xe = epool.tile([P, w], BF16, tag="xe", padded_shape=[P, WMAX])
                    nc.scalar.activation(out=xe[:], in_=xc[:], func=AF.Exp,
                                         scale=-1.0)
                    h = epool.tile([P, w], BF16, tag="h", padded_shape=[P, WMAX])
                    nc.vector.tensor_tensor(out=h[:], in0=xm[:], in1=xe[:],
                                            op=ALU.add)
                    if not shift:
                        h2 = epool.tile([P, w], BF16, tag="h2",
                                        padded_shape=[P, WMAX])
                        nc.vector.tensor_tensor(
                            out=h2[:], in0=h[:],
                            in1=mone_sb[:, :1].to_broadcast([P, w]), op=ALU.add)
                        h = h2
                    out_cb(s, ts, w, c0, h)
                    if mid_cb is not None and s == s_ag:
                        mid_cb()

            # ---- layer 0 aggregate -> h1T -> z1/rb1T ----
            def l0_out(s, ts, w, c0, h):
                for ti, tt in enumerate(ts):
                    z1ps = pz.tile([P, P], F32, tag="zps")
                    nc.tensor.matmul(out=z1ps[:],
                                     lhsT=h[:, ti * P:(ti + 1) * P],
                                     rhs=w1l_sb[:], start=True, stop=True)
                    z1sb = zpool.tile([P, P], BF16, tag="zsb")
                    nc.vector.tensor_copy(out=z1sb[:], in_=z1ps[:])
                    z_to_cc(z1sb, tt, cc1)
                r1ps = pr.tile([P, w], F32, tag="rps", padded_shape=[P, WMAX])
                nc.tensor.matmul(out=r1ps[:], lhsT=w1r_sb[:], rhs=h[:],
                                 start=True, stop=True)
                nc.vector.tensor_tensor(out=rb1_sb[:, c0:c0 + w], in0=r1ps[:],
                                        in1=b1_sb[:, :1].to_broadcast([P, w]),
                                        op=ALU.add)

            # prologue: fill lo lookahead, then a short hi lead
            pump_lo(NBUF_LO)
            pump_hi(HI_LOOK)

            run_layer(0, rb0_sb, l0_out, mid_cb=lambda: ag(cc1, 0))
            ag(cc1, 1)

            # ---- layer 1 aggregate -> output ----
            def l1_out(s, ts, w, c0, h):
                nc.scalar.dma_start(out=out_d[:, c0:c0 + w], in_=h[:])

            run_layer(1, rb1_sb, l1_out)
            pa_cm.__exit__(None, None, None)

    nc.compile()
    return nc


# ---------------------------------------------------------------------------
# entry point
# ---------------------------------------------------------------------------

def _bf16(a):
    return np.asarray(a, np.float32).astype(ml_dtypes.bfloat16)


def _prepare(x, knn_edge_index, W_l0, b_l0, W_r0, W_l1, b_l1, W_r1,
             NC=8, tiles_per_st=4):
    x = np.asarray(x, np.float32)
    e = np.asarray(knn_edge_index)
    in_dim, N = x.shape
    src, dst = e[0].astype(np.int64), e[1].astype(np.int64)
    meta = build_meta(N, NC, dst, src, tiles_per_st)
    npc, NPAD = meta["npc"], meta["NPAD"]
    GC = (in_dim + P - 1) // P
    GPAD = GC * P

    W_l1 = np.asarray(W_l1, np.float32)
    W_r1 = np.asarray(W_r1, np.float32)
    b1 = np.asarray(b_l1, np.float32)
    if meta["shift_ok"]:
        b1 = b1 - W_l1.sum(axis=1) - W_r1.sum(axis=1)

    w0l = np.zeros((GPAD, P), np.float32); w0l[:in_dim] = np.asarray(W_l0).T
    w0r = np.zeros((GPAD, P), np.float32); w0r[:in_dim] = np.asarray(W_r0).T
    shared = {
        "W0lT": _bf16(w0l), "W0rT": _bf16(w0r),
        "W1lT": _bf16(np.ascontiguousarray(W_l1.T)),
        "W1rT": _bf16(np.ascontiguousarray(W_r1.T)),
        "b0col": np.asarray(b_l0, np.float32).reshape(P, 1),
        "b1col": b1.reshape(P, 1),
        "iota": _bf16(np.broadcast_to(np.arange(P, dtype=np.float32), (P, P))),
        "ident": _bf16(np.eye(P, dtype=np.float32)),
    }
    in_maps = []
    for c in range(NC):
        xp = np.zeros((GPAD, NPAD), np.float32)
        xp[:in_dim, :npc] = x[:, c * npc:(c + 1) * npc]
        m = dict(shared)
        m["x_pad"] = _bf16(xp)
        m["invt"] = np.ascontiguousarray(
            np.broadcast_to(meta["inv"][c], (P, NPAD)))
        m["idx16"] = np.ascontiguousarray(meta["idx_slab"][c])
        m["gcnts"] = np.ascontiguousarray(
            np.broadcast_to(meta["counts"][c], (P, meta["NG"])).astype(np.int32))
        m["dstid"] = _bf16(meta["dstid_slab"][c])
        in_maps.append(m)
    return meta, in_dim, in_maps


def run(inputs, NC=8, tiles_per_st=4, trace=False, **run_kwargs):
    meta, in_dim, in_maps = _prepare(**inputs, NC=NC, tiles_per_st=tiles_per_st)
    nc = build_kernel(meta, in_dim, NC)
    res = bass_utils.run_bass_kernel_spmd(
        nc, in_maps, core_ids=list(range(NC)), trace=trace, **run_kwargs)
    npc = meta["npc"]
    sub = 1.0 if meta["shift_ok"] else 0.0
    out = np.concatenate(
        [res.results[c]["outT"][:, :npc].T.astype(np.float32) - sub
         for c in range(NC)], axis=0)
    return np.ascontiguousarray(out), res


def kernel(**inputs) -> np.ndarray:
    out, _ = run(inputs)
    return out


# revision 17
# speedup vs baseline: 1.3942x; 1.1816x over previous
"""Trainium2 Bass kernel for nn_CellEncoder (2-layer GraphSAGE, mean aggregation).

Strategy (8 NeuronCores, SPMD, node-partitioned), v5:
  - Core c owns nodes [c*npc, (c+1)*npc).  Aggregation is linear, so the
    dense transform is applied FIRST: z = h @ W_l.T reduces gather width
    from in_dim (1000) to emb (128) values per edge.  All tables/operands
    are bf16 (PSUM accumulation fp32); tolerance is 2e-2, bf16 ~5e-3.
  - The z table is split into SEGS=3 row segments, each AllGathered
    separately as soon as its producer rows are ready: segment-0 gathers
    start while phase A is still computing later segments' z, and the
    layer-1 boundary only stalls on the (small) last segment's AllGather.
  - Edges grouped by (dst tile, src segment); slots packed into 128-slot
    chunks.  One dma_gather per (tile, segment) (<= 8 chunks, 1024 idxs,
    single_packet fast path) on the least-chunk-loaded of the 4 SWDGE
    queues (a plain round-robin aliases with piece sizes and starves
    queues).  Padding slots are NEGATIVE indices at the gather tail (the
    DMA skips them) and num_idxs_reg is reg_load-ed from a per-core count
    table, so the descriptor stream is exactly the core's edge count.
  - The int16 index slab (shared by both layers -- same edges) is loaded
    once into SBUF.  Gather buffers are fixed SBUF slabs with manual
    cycling per segment stream; earlier segments get deeper lookahead.
  - One-hot scatter matrices S[e,d] = (dst(e)==d) built on DVE per tile
    with a batched is_equal against a materialized iota (contiguous in1).
    PE accumulates aggT[f,d] += G_chunk.T @ S_chunk in PSUM per group of
    4 tiles.  Skipped padding slots hold zeros/stale rows (finite), S=0.
  - Phase A computes z0 feature-major with stationary W_l0, transposes
    per tile on the PE against identity, and only then computes the r0
    term -- z (which gates the AllGathers) finishes as early as possible.
  - ELU's "-1" is folded out: the device computes h~ = elu(x)+1; the next
    layer's bias is adjusted on the host and the host subtracts 1 from
    the final output (requires min in-degree >= 1, checked on host).
    exp(min(x,0)) is computed as Exp(-Relu(-x)) on the scalar engine.

kernel(**inputs) takes FULL inputs, shards internally, runs one NEFF on
cores 0-7 via bass_utils.run_bass_kernel_spmd, returns the full output.
"""
import os
import sys

import numpy as np

for _p in ("/opt/trn_rl_repo", "/root/.axon_site/_ro/trn_rl_repo"):
    if os.path.isdir(_p) and _p not in sys.path:
        sys.path.append(_p)

import ml_dtypes

import concourse.bass as bass
import concourse.bacc as bacc
import concourse.mybir as mybir
import concourse.tile as tile
from concourse import bass_utils

P = 128
F32 = mybir.dt.float32
BF16 = mybir.dt.bfloat16
I16 = mybir.dt.int16
I32 = mybir.dt.int32
AF = mybir.ActivationFunctionType
ALU = mybir.AluOpType

SCRATCH = 49152    # SWDGE ring: scratch//16 descriptors per queue
PIECE = 8          # max chunks per gather (1024 idxs, single_packet path)
SEGS = 3
NBUFS = (10, 7, 6)     # per-segment gather tile buffers in flight
LOOKS = (9, 5, 3)      # per-segment issue leads (tiles); < NBUFS[k]
L1_GATES = (28, 40, None)  # layer-0 tile from which layer-1 seg-k issues


def build_meta(N, NC, dst, src, tiles_per_st):
    """Static chunk structure (shared across cores; max-over-core sizes),
    per-core gather-index / dst-id slabs with -1 padding tails, and per-core
    per-gather valid-index counts."""
    npc = N // NC
    half = npc // 2
    TPC = (npc + P - 1) // P
    NPAD = TPC * P
    NST = (TPC + tiles_per_st - 1) // tiles_per_st

    b1 = (npc + SEGS - 1) // SEGS
    segb = [min(k * b1, npc) for k in range(SEGS + 1)]
    segsz = [segb[k + 1] - segb[k] for k in range(SEGS)]
    assert all(NC * s <= 32768 for s in segsz)

    c = dst // npc
    d = (dst - c * npc).astype(np.int64)
    t = d // P
    did = d % P
    sc = src // npc
    sp = src - sc * npc
    tb = np.minimum(sp // b1, SEGS - 1).astype(np.int64)
    row = sc * np.array(segsz)[tb] + (sp - np.array(segb)[tb])
    assert row.max() < 32768

    nseg = np.zeros((NC, TPC, SEGS), np.int64)
    np.add.at(nseg, (c, t, tb), 1)
    KS = np.maximum(1, (nseg.max(axis=0) + P - 1) // P)  # [TPC, SEGS]
    assert KS.max() <= PIECE, KS.max()

    Ktot = KS.sum(axis=1)
    chunk_base = np.concatenate([[0], np.cumsum(Ktot)])
    NCHUNK = int(chunk_base[-1])
    KMAXS = [int(KS[:, k].max()) for k in range(SEGS)]

    st_tiles = [list(range(s * tiles_per_st, min((s + 1) * tiles_per_st, TPC)))
                for s in range(NST)]

    # one gather per (tile, segment)
    idx_off16 = np.zeros((TPC, SEGS), np.int64)
    gidx = np.zeros((TPC, SEGS), np.int64)
    off = 0
    g = 0
    for tt in range(TPC):
        for k in range(SEGS):
            idx_off16[tt, k] = off
            gidx[tt, k] = g
            off += int(KS[tt, k]) * P // 16
            g += 1
    NIDX16 = int(off)
    NG = int(g)

    idx_slab = np.zeros((NC, P, NIDX16), np.int16)
    dstid_slab = np.full((NC, P, NCHUNK), -1.0, np.float32)
    counts = np.zeros((NC, NG), np.int32)
    cnt = np.zeros((NC, NPAD), np.int64)

    order = np.lexsort((tb, t, c))
    co, to, tbo = c[order], t[order], tb[order]
    rowo, dido, do_ = row[order], did[order], d[order]
    np.add.at(cnt, (co, do_), 1)

    key = (co * TPC + to) * SEGS + tbo
    bounds = np.concatenate([[0], np.nonzero(np.diff(key))[0] + 1, [len(key)]])
    filled = np.zeros((NC, TPC, SEGS), bool)
    for bi in range(len(bounds) - 1):
        lo_, hi_ = int(bounds[bi]), int(bounds[bi + 1])
        if lo_ == hi_:
            continue
        cc, tt, kk = int(co[lo_]), int(to[lo_]), int(tbo[lo_])
        n = hi_ - lo_
        K = int(KS[tt, kk])
        v = np.full(K * P, -1, np.int16)
        v[:n] = rowo[lo_:hi_]
        counts[cc, gidx[tt, kk]] = n
        filled[cc, tt, kk] = True
        o16 = int(idx_off16[tt, kk])
        w = v.reshape(K * P // 16, 16).T
        idx_slab[cc, :, o16:o16 + K * P // 16] = np.tile(w, (8, 1))
        ch0 = int(chunk_base[tt]) + int(KS[tt, :kk].sum())
        local = np.arange(n)
        dstid_slab[cc, local % P, ch0 + local // P] = dido[lo_:hi_]

    # empty (core,tile,seg): one dummy valid idx (row 0; dstid -1 -> S = 0)
    for cc, tt, kk in zip(*np.nonzero(~filled)):
        K = int(KS[tt, kk])
        v = np.full(K * P, -1, np.int16)
        v[0] = 0
        counts[cc, gidx[tt, kk]] = 1
        o16 = int(idx_off16[tt, kk])
        w = v.reshape(K * P // 16, 16).T
        idx_slab[cc, :, o16:o16 + K * P // 16] = np.tile(w, (8, 1))

    inv = (1.0 / np.maximum(cnt, 1)).astype(np.float32)
    shift_ok = bool(cnt[:, :npc].min() >= 1)

    return dict(
        npc=npc, half=half, TPC=TPC, NPAD=NPAD, NST=NST, st_tiles=st_tiles,
        segb=segb, segsz=segsz,
        KS=KS.astype(int), KMAXS=KMAXS,
        chunk_base=[int(v) for v in chunk_base], NCHUNK=NCHUNK,
        idx_off16=idx_off16.astype(int), gidx=gidx.astype(int),
        NIDX16=NIDX16, NG=NG,
        idx_slab=idx_slab, dstid_slab=dstid_slab, counts=counts, inv=inv,
        shift_ok=shift_ok,
    )


# ---------------------------------------------------------------------------
# device kernel builder
# ---------------------------------------------------------------------------

def build_kernel(meta, in_dim, NC):
    npc = meta["npc"]
    TPC, NPAD, NST = meta["TPC"], meta["NPAD"], meta["NST"]
    NCHUNK, NIDX16, NG = meta["NCHUNK"], meta["NIDX16"], meta["NG"]
    KS, KMAXS = meta["KS"], meta["KMAXS"]
    segb, segsz = meta["segb"], meta["segsz"]
    chunk_base = meta["chunk_base"]
    idx_off16, gidx = meta["idx_off16"], meta["gidx"]
    shift = meta["shift_ok"]
    GC = (in_dim + P - 1) // P
    GPAD = GC * P
    WMAX = max(len(ts) for ts in meta["st_tiles"]) * P
    KT2MAX = int(max(KS[t].sum() for t in range(TPC)))
    qload = [0, 0, 0, 0]

    nc = bacc.Bacc("TRN2", target_bir_lowering=False, debug=False,
                   enable_asserts=False, num_devices=NC,
                   dynamic_dma_scratch_size=SCRATCH, num_swdge_queues=4)

    x_d = nc.dram_tensor("x_pad", [GPAD, NPAD], BF16, kind="ExternalInput").ap()
    w0l_d = nc.dram_tensor("W0lT", [GPAD, P], BF16, kind="ExternalInput").ap()
    w0r_d = nc.dram_tensor("W0rT", [GPAD, P], BF16, kind="ExternalInput").ap()
    w1l_d = nc.dram_tensor("W1lT", [P, P], BF16, kind="ExternalInput").ap()
    w1r_d = nc.dram_tensor("W1rT", [P, P], BF16, kind="ExternalInput").ap()
    b0_d = nc.dram_tensor("b0col", [P, 1], F32, kind="ExternalInput").ap()
    b1_d = nc.dram_tensor("b1col", [P, 1], F32, kind="ExternalInput").ap()
    inv_d = nc.dram_tensor("invt", [P, NPAD], F32, kind="ExternalInput").ap()
    idx_d = nc.dram_tensor("idx16", [P, NIDX16], I16, kind="ExternalInput").ap()
    cnts_d = nc.dram_tensor("gcnts", [P, NG], I32, kind="ExternalInput").ap()
    iota_d = nc.dram_tensor("iota", [P, P], BF16, kind="ExternalInput").ap()
    ident_d = nc.dram_tensor("ident", [P, P], BF16, kind="ExternalInput").ap()
    dst_d = nc.dram_tensor("dstid", [P, NCHUNK], BF16, kind="ExternalInput").ap()
    out_d = nc.dram_tensor("outT", [P, NPAD], BF16, kind="ExternalOutput").ap()

    with tile.TileContext(nc, num_cores=NC) as tc:
        with (
            tc.tile_pool(name="const", bufs=1) as cpool,
            tc.tile_pool(name="slab", bufs=1) as slab,
            tc.tile_pool(name="zp", bufs=4) as zpool,
            tc.tile_pool(name="ep", bufs=2) as epool,
            tc.tile_pool(name="sp", bufs=3) as spool,
            tc.tile_pool(name="ip", bufs=2) as ipool,
            tc.tile_pool(name="pz", bufs=3, space="PSUM") as pz,
            tc.tile_pool(name="pr", bufs=2, space="PSUM") as pr,
            tc.tile_pool(name="dram", bufs=1, space="DRAM") as dram,
        ):
            # ---- constants ----
            w0l_sb = cpool.tile([P, GC * P], BF16)
            w0r_sb = cpool.tile([P, GC * P], BF16)
            for gc in range(GC):
                nc.sync.dma_start(out=w0l_sb[:, gc * P:(gc + 1) * P],
                                  in_=w0l_d[gc * P:(gc + 1) * P, :])
                nc.sync.dma_start(out=w0r_sb[:, gc * P:(gc + 1) * P],
                                  in_=w0r_d[gc * P:(gc + 1) * P, :])
            w1l_sb = cpool.tile([P, P], BF16)
            nc.sync.dma_start(out=w1l_sb[:], in_=w1l_d[:])
            w1r_sb = cpool.tile([P, P], BF16)
            nc.sync.dma_start(out=w1r_sb[:], in_=w1r_d[:])
            b0_sb = cpool.tile([P, 1], F32)
            nc.sync.dma_start(out=b0_sb[:], in_=b0_d[:])
            b1_sb = cpool.tile([P, 1], F32)
            nc.sync.dma_start(out=b1_sb[:], in_=b1_d[:])
            mone_sb = cpool.tile([P, 1], BF16)
            nc.vector.memset(mone_sb[:], -1.0)
            # iota materialized at full S width: contiguous is_equal operand
            iota_sb = cpool.tile([P, KT2MAX * P], BF16)
            for k in range(KT2MAX):
                nc.sync.dma_start(out=iota_sb[:, k * P:(k + 1) * P], in_=iota_d[:])
            ident_sb = cpool.tile([P, P], BF16)
            nc.sync.dma_start(out=ident_sb[:], in_=ident_d[:])
            dst_sb = cpool.tile([P, NCHUNK], BF16)
            nc.sync.dma_start(out=dst_sb[:], in_=dst_d[:])
            idx_sb = cpool.tile([P, NIDX16], I16)
            nc.sync.dma_start(out=idx_sb[:], in_=idx_d[:])
            cnts_sb = cpool.tile([P, NG], I32)
            nc.sync.dma_start(out=cnts_sb[:], in_=cnts_d[:])

            rb0_sb = slab.tile([P, NPAD], BF16)
            rb1_sb = slab.tile([P, NPAD], BF16)

            # gather slabs (manual buffer cycling); zeroed once on the (idle
            # through phase A) Pool engine so skipped padding slots never
            # feed NaN/Inf garbage into the PE
            gseg_sb = []
            for k in range(SEGS):
                # explicit tags: same-line tile() calls share an auto-tag,
                # which in a bufs=1 pool would serialize the slabs' lifetimes
                t_ = cpool.tile([P, NBUFS[k] * KMAXS[k] * P], BF16,
                                tag=f"gseg{k}")
                nc.gpsimd.memset(t_[:], 0.0)
                gseg_sb.append(t_)

            greg = nc.gpsimd.alloc_register("gcnt")

            # ---- collective buffers ----
            def cc_set(nm):
                ins_ = [dram.tile([segsz[k], P], BF16, name=f"cci{k}_{nm}",
                                  tag=f"cci{k}_{nm}")
                        for k in range(SEGS)]
                outs_ = [dram.tile([NC * segsz[k], P], BF16, addr_space="Shared",
                                   name=f"cco{k}_{nm}", tag=f"cco{k}_{nm}")
                         for k in range(SEGS)]
                return ins_, outs_

            cc0i, cc0o = cc_set("0")
            cc1i, cc1o = cc_set("1")
            rg = [list(range(NC))]

            def z_to_cc(z_sb, tt, cci):
                r0, r1 = tt * P, min(tt * P + P, npc)
                for k in range(SEGS):
                    lo_s, hi_s = max(r0, segb[k]), min(r1, segb[k + 1])
                    if hi_s > lo_s:
                        nc.scalar.dma_start(
                            out=cci[k][lo_s - segb[k]:hi_s - segb[k], :],
                            in_=z_sb[lo_s - r0:hi_s - r0, :])

            def ag(cci, cco, k):
                nc.gpsimd.collective_compute(
                    "AllGather", ALU.bypass, replica_groups=rg,
                    ins=[cci[k][:].opt()], outs=[cco[k][:].opt()])

            # ---- phase A: z0 (fm + PE transpose) first, then r0 ----
            pa_groups = [list(range(s * 2, min(s * 2 + 2, TPC)))
                         for s in range((TPC + 1) // 2)]
            # AG for segment k fires once all z rows < segb[k+1] are written
            ag_after = {}
            for k in range(SEGS - 1):
                g_ = next(i for i, gts in enumerate(pa_groups)
                          if (gts[-1] + 1) * P >= segb[k + 1])
                ag_after.setdefault(g_, []).append(k)
            with (
                tc.tile_pool(name="xp", bufs=2) as xpool,
                tc.tile_pool(name="pf", bufs=2, space="PSUM") as pf,
            ):
                for s, ts in enumerate(pa_groups):
                    w = len(ts) * P
                    c0 = ts[0] * P
                    xg = xpool.tile([P, GC * w], BF16, tag="xg",
                                    padded_shape=[P, GC * 2 * P])
                    nc.sync.dma_start(
                        out=xg[:].rearrange("p (gc j) -> p gc j", gc=GC),
                        in_=x_d[:, c0:c0 + w].rearrange("(gc p) j -> p gc j", p=P))
                    zfm = pf.tile([P, w], F32, tag="zfm", padded_shape=[P, 2 * P])
                    for gc in range(GC):
                        nc.tensor.matmul(out=zfm[:],
                                         lhsT=w0l_sb[:, gc * P:(gc + 1) * P],
                                         rhs=xg[:, gc * w:(gc + 1) * w],
                                         start=(gc == 0), stop=(gc == GC - 1))
                    zfm_sb = zpool.tile([P, w], BF16, tag="zfm_sb",
                                        padded_shape=[P, 2 * P])
                    nc.vector.tensor_copy(out=zfm_sb[:], in_=zfm[:])
                    for ti, tt in enumerate(ts):
                        zT = pz.tile([P, P], F32, tag="zps")
                        nc.tensor.matmul(out=zT[:],
                                         lhsT=zfm_sb[:, ti * P:(ti + 1) * P],
                                         rhs=ident_sb[:], start=True, stop=True)
                        z0sb = zpool.tile([P, P], BF16, tag="zsb")
                        nc.vector.tensor_copy(out=z0sb[:], in_=zT[:])
                        z_to_cc(z0sb, tt, cc0i)
                    r0ps = pr.tile([P, w], F32, tag="rps", padded_shape=[P, WMAX])
                    for gc in range(GC):
                        nc.tensor.matmul(out=r0ps[:],
                                         lhsT=w0r_sb[:, gc * P:(gc + 1) * P],
                                         rhs=xg[:, gc * w:(gc + 1) * w],
                                         start=(gc == 0), stop=(gc == GC - 1))
                    nc.vector.tensor_tensor(out=rb0_sb[:, c0:c0 + w], in0=r0ps[:],
                                            in1=b0_sb[:, :1].to_broadcast([P, w]),
                                            op=ALU.add)
                    for k in ag_after.get(s, []):
                        ag(cc0i, cc0o, k)
            ag(cc0i, cc0o, SEGS - 1)

            # pa opens after phase A's pf pool closes (PSUM: 8 banks, 7+7)
            pa_cm = tc.tile_pool(name="pa", bufs=2, space="PSUM")
            pa = pa_cm.__enter__()

            # ---- aggregation machinery -------------------------------------
            tabs = (cc0o, cc1o)
            state = [0] * SEGS  # issue cursor per segment stream, in steps

            def issue_gather(k, step):
                layer, tt = divmod(step, TPC)
                buf = step % NBUFS[k]
                base = buf * KMAXS[k] * P
                K = int(KS[tt, k])
                g = int(gidx[tt, k])
                o16 = int(idx_off16[tt, k])
                q = min(range(4), key=lambda i: qload[i])
                qload[q] += K
                nc.gpsimd.reg_load(greg, cnts_sb[0:1, g:g + 1])
                nc.gpsimd.dma_gather(
                    out_ap=gseg_sb[k][:, base:base + K * P]
                    .rearrange("p (kk e) -> p kk e", e=P),
                    in_ap=tabs[layer][k][:],
                    idxs_ap=idx_sb[:, o16:o16 + K * 8],
                    num_idxs=K * P, num_idxs_reg=greg, elem_size=P,
                    single_packet=True, queue_num=q)

            def pump(k, limit):
                while state[k] < min(limit, 2 * TPC):
                    issue_gather(k, state[k])
                    state[k] += 1

            def aggregate_tile(layer, tt, aggps, ti):
                nch = int(KS[tt].sum())
                cb0 = chunk_base[tt]
                s_sb = spool.tile([P, nch * P], BF16, tag="ssb",
                                  padded_shape=[P, KT2MAX * P])
                nc.vector.tensor_tensor(
                    out=s_sb[:].rearrange("p (n e) -> p n e", e=P),
                    in0=dst_sb[:, cb0:cb0 + nch].unsqueeze(2)
                    .to_broadcast([P, nch, P]),
                    in1=iota_sb[:, :nch * P].rearrange("p (n e) -> p n e", e=P),
                    op=ALU.is_equal)
                step = layer * TPC + tt
                j = 0
                for k in range(SEGS):
                    base = (step % NBUFS[k]) * KMAXS[k] * P
                    for jj in range(int(KS[tt, k])):
                        g_ap = gseg_sb[k][:, base + jj * P:base + (jj + 1) * P]
                        nc.tensor.matmul(out=aggps[:, ti * P:(ti + 1) * P],
                                         lhsT=g_ap,
                                         rhs=s_sb[:, j * P:(j + 1) * P],
                                         start=(j == 0), stop=(j == nch - 1))
                        j += 1

            s_ag = ((npc // 2 + P - 1) // P - 1) // len(meta["st_tiles"][0])
            # layer-1 AG trigger groups: z1 rows < segb[k+1] complete
            ag1_after = {}
            for k in range(SEGS - 1):
                g_ = next(i for i, gts in enumerate(meta["st_tiles"])
                          if (gts[-1] + 1) * P >= segb[k + 1])
                ag1_after.setdefault(g_, []).append(k)

            def run_layer(layer, rb_slab, out_cb, mid_cb=None):
                for s, ts in enumerate(meta["st_tiles"]):
                    w = len(ts) * P
                    c0 = ts[0] * P
                    aggps = pa.tile([P, w], F32, tag="aggps",
                                    padded_shape=[P, WMAX])
                    for ti, tt in enumerate(ts):
                        step = layer * TPC + tt
                        for k in reversed(range(SEGS)):
                            limit = step + LOOKS[k] + 1
                            if layer == 0:
                                if L1_GATES[k] is None:
                                    # its AG trigger comes after this loop
                                    limit = min(limit, TPC)
                                elif tt < L1_GATES[k]:
                                    limit = min(limit, TPC)
                            pump(k, limit)
                        aggregate_tile(layer, tt, aggps, ti)
                    invt = ipool.tile([P, w], F32, tag="invt",
                                      padded_shape=[P, WMAX])
                    nc.sync.dma_start(out=invt[:], in_=inv_d[:, c0:c0 + w])
                    x2 = epool.tile([P, w], BF16, tag="x2", padded_shape=[P, WMAX])
                    nc.vector.tensor_tensor(out=x2[:], in0=aggps[:],
                                            in1=invt[:], op=ALU.mult)
                    x3 = epool.tile([P, w], BF16, tag="x3", padded_shape=[P, WMAX])
                    nc.vector.tensor_tensor(out=x3[:], in0=x2[:],
                                            in1=rb_slab[:, c0:c0 + w], op=ALU.add)
                    xm = epool.tile([P, w], BF16, tag="xm", padded_shape=[P, WMAX])
                    nc.scalar.activation(out=xm[:], in_=x3[:], func=AF.Relu)
                    # exp(min(x,0)) = Exp(-Relu(-x)), both on the scalar engine
                    xc = epool.tile([P, w], BF16, tag="xc", padded_shape=[P, WMAX])
                    nc.scalar.activation(out=xc[:], in_=x3[:], func=AF.Relu,
                                         scale=-1.0)
                    xe = epool.tile([P, w], BF16, tag="xe", padded_shape=[P, WMAX])
                    nc.scalar.activation(out=xe[:], in_=xc[:], func=AF.Exp,
                                         scale=-1.0)
                    h = epool.tile([P, w], BF16, tag="h", padded_shape=[P, WMAX])
                    nc.vector.tensor_tensor(out=h[:], in0=xm[:], in1=xe[:],
                                            op=ALU.add)
                    if not shift:
                        h2 = epool.tile([P, w], BF16, tag="h2",
                                        padded_shape=[P, WMAX])
                        nc.vector.tensor_tensor(
                            out=h2[:], in0=h[:],
                            in1=mone_sb[:, :1].to_broadcast([P, w]), op=ALU.add)
                        h = h2
                    out_cb(s, ts, w, c0, h)
                    if mid_cb is not None:
                        mid_cb(s)

            # ---- layer 0 aggregate -> h1T -> z1/rb1T ----
            def l0_out(s, ts, w, c0, h):
                for ti, tt in enumerate(ts):
                    z1ps = pz.tile([P, P], F32, tag="zps")
                    nc.tensor.matmul(out=z1ps[:],
                                     lhsT=h[:, ti * P:(ti + 1) * P],
                                     rhs=w1l_sb[:], start=True, stop=True)
                    z1sb = zpool.tile([P, P], BF16, tag="zsb")
                    nc.vector.tensor_copy(out=z1sb[:], in_=z1ps[:])
                    z_to_cc(z1sb, tt, cc1i)
                r1ps = pr.tile([P, w], F32, tag="rps", padded_shape=[P, WMAX])
                nc.tensor.matmul(out=r1ps[:], lhsT=w1r_sb[:], rhs=h[:],
                                 start=True, stop=True)
                nc.vector.tensor_tensor(out=rb1_sb[:, c0:c0 + w], in0=r1ps[:],
                                        in1=b1_sb[:, :1].to_broadcast([P, w]),
                                        op=ALU.add)

            # prologue: deepest stream first, then shorter leads
            pump(0, NBUFS[0])
            for k in range(1, SEGS):
                pump(k, LOOKS[k])

            def l0_mid(s):
                for k in ag1_after.get(s, []):
                    ag(cc1i, cc1o, k)

            run_layer(0, rb0_sb, l0_out, mid_cb=l0_mid)
            ag(cc1i, cc1o, SEGS - 1)

            # ---- layer 1 aggregate -> output ----
            def l1_out(s, ts, w, c0, h):
                nc.scalar.dma_start(out=out_d[:, c0:c0 + w], in_=h[:])

            run_layer(1, rb1_sb, l1_out)
            pa_cm.__exit__(None, None, None)

    nc.compile()
    return nc


# ---------------------------------------------------------------------------
# entry point
# ---------------------------------------------------------------------------

def _bf16(a):
    return np.asarray(a, np.float32).astype(ml_dtypes.bfloat16)


def _prepare(x, knn_edge_index, W_l0, b_l0, W_r0, W_l1, b_l1, W_r1,
             NC=8, tiles_per_st=4):
    x = np.asarray(x, np.float32)
    e = np.asarray(knn_edge_index)
    in_dim, N = x.shape
    src, dst = e[0].astype(np.int64), e[1].astype(np.int64)
    meta = build_meta(N, NC, dst, src, tiles_per_st)
    npc, NPAD = meta["npc"], meta["NPAD"]
    GC = (in_dim + P - 1) // P
    GPAD = GC * P

    W_l1 = np.asarray(W_l1, np.float32)
    W_r1 = np.asarray(W_r1, np.float32)
    b1 = np.asarray(b_l1, np.float32)
    if meta["shift_ok"]:
        b1 = b1 - W_l1.sum(axis=1) - W_r1.sum(axis=1)

    w0l = np.zeros((GPAD, P), np.float32); w0l[:in_dim] = np.asarray(W_l0).T
    w0r = np.zeros((GPAD, P), np.float32); w0r[:in_dim] = np.asarray(W_r0).T
    shared = {
        "W0lT": _bf16(w0l), "W0rT": _bf16(w0r),
        "W1lT": _bf16(np.ascontiguousarray(W_l1.T)),
        "W1rT": _bf16(np.ascontiguousarray(W_r1.T)),
        "b0col": np.asarray(b_l0, np.float32).reshape(P, 1),
        "b1col": b1.reshape(P, 1),
        "iota": _bf16(np.broadcast_to(np.arange(P, dtype=np.float32), (P, P))),
        "ident": _bf16(np.eye(P, dtype=np.float32)),
    }
    in_maps = []
    for c in range(NC):
        xp = np.zeros((GPAD, NPAD), np.float32)
        xp[:in_dim, :npc] = x[:, c * npc:(c + 1) * npc]
        m = dict(shared)
        m["x_pad"] = _bf16(xp)
        m["invt"] = np.ascontiguousarray(
            np.broadcast_to(meta["inv"][c], (P, NPAD)))
        m["idx16"] = np.ascontiguousarray(meta["idx_slab"][c])
        m["gcnts"] = np.ascontiguousarray(
            np.broadcast_to(meta["counts"][c], (P, meta["NG"])).astype(np.int32))
        m["dstid"] = _bf16(meta["dstid_slab"][c])
        in_maps.append(m)
    return meta, in_dim, in_maps


def run(inputs, NC=8, tiles_per_st=4, trace=False, **run_kwargs):
    meta, in_dim, in_maps = _prepare(**inputs, NC=NC, tiles_per_st=tiles_per_st)
    nc = build_kernel(meta, in_dim, NC)
    res = bass_utils.run_bass_kernel_spmd(
        nc, in_maps, core_ids=list(range(NC)), trace=trace, **run_kwargs)
    npc = meta["npc"]
    sub = 1.0 if meta["shift_ok"] else 0.0
    out = np.concatenate(
        [res.results[c]["outT"][:, :npc].T.astype(np.float32) - sub
         for c in range(NC)], axis=0)
    return np.ascontiguousarray(out), res


def kernel(**inputs) -> np.ndarray:
    out, _ = run(inputs)
    return out
